# revision 25
# baseline (speedup 1.0000x reference)
"""Trainium2 Bass kernel for nn_EssentialMatrixEstimator.

Distribution: data-parallel over the N=3072 rows of Pc across 8 cores
(384 rows each).

v2 redesign vs the two-phase baseline:
  * The transposed shard W^T is shipped from host (pure layout prep), killing
    all 72 on-device PE transposes.
  * Monomials are PRE-CENTERED on host with fixed (c0, s0) derived from K
    (grid centroid / RMS), so a SINGLE well-conditioned 6x6 Gram C' suffices:
    the Hartley-centered Gram is recovered exactly as C2 = L1 C' L2^T where
    L1/L2 are 6x6 monomial shift/scale transforms built from C''s own
    moments (row/col 5). One AllGather (column top-3 partials) + one
    AllReduce (6x6 Gram) are the only collectives.
  * Gram computed as B^T = sum_j M2_j^T W^T_j (24 wide fp32 matmuls,
    stream 384) then C' = M1^T B (3 small matmuls after 3 PE transposes).
  * Masking fused: pre-AG  w = X * (X >= max(rowthr, T0)) (2 passes,
    split DVE/GpSimd, hidden under the AllGather); post-AG one fused
    scalar_tensor_tensor pass  w = (w >= colthr) * w  per 128-col block,
    pipelined into the Gram matmuls.
  * Power iterations: rescaled repeated squaring M^50 = 2*(2*(M32@M16)@M2)
    (M^48/M^64 were checked and are NOT converged enough - keep 50).
Validated against reference (rel err ~2e-5; tolerance 2e-2).
"""

import os

os.environ.setdefault("JAX_PLATFORMS", "axon")

import numpy as np

import concourse.bass as bass
import concourse.bass_isa as bass_isa
import concourse.mybir as mybir
import concourse.bacc as bacc
import concourse.tile as tile

NCORES = 8
N = 3072
SH = N // NCORES          # 384 rows per core
RT = SH // 128            # 3 row tiles per core
CB = N // 128             # 24 column blocks
F32 = mybir.dt.float32
AF = mybir.ActivationFunctionType
OP = mybir.AluOpType
AX = mybir.AxisListType

EPS = 1e-8
SQRT2 = 1.4142135623730951
INV_SQRT3 = 1.0 / 1.7320508075688772
T0 = float(np.nextafter(np.float32(0.01), np.float32(1)))  # x > 0.01 == x >= T0
H, W = 64, 64

DVE_BLOCKS = 15  # mask blocks on DVE; rest on gpsimd

# cpack const layout (tensor [9, 36]): column ranges
C_I9H = 0      # I9 * 0.5          [9, 9]
C_ET69 = 9     # E^T selector      [6, 9]
C_I3 = 18      # I3                [3, 3]
C_V09 = 21     # full(1/3)         [9, 1]
C_V06 = 22     # full(1/sqrt3)     [6, 1]
C_SEL1 = 23    # [I3 | 0]          [3, 6]
C_SEL2 = 29    # [0 | I3]          [3, 6]
C_E5 = 35      # e5 selector       [6, 1]

PAIRS = [(0, 0), (0, 1), (0, 2), (1, 1), (1, 2), (2, 2)]


def _pidx():
    d = {}
    for i, (a, b) in enumerate(PAIRS):
        d[(a, b)] = i
        d[(b, a)] = i
    return d


def host_constants(K):
    """Pre-centered monomial matrix + packed constants + (c0, s0)."""
    idx = np.arange(H * W, dtype=np.float32)
    pix = np.stack([idx % np.float32(W), np.floor(idx / np.float32(W))], -1)
    K_inv = np.linalg.inv(np.asarray(K, np.float32)).astype(np.float32)
    p1h = np.concatenate([pix[:N], np.ones((N, 1), np.float32)], -1)
    pts = (p1h @ K_inv.T)[:, :2].astype(np.float32)  # same grid both sides
    c0 = pts.mean(0).astype(np.float32)
    pc = pts - c0
    s0 = np.float32(SQRT2) / np.float32(np.sqrt((pc ** 2).sum(-1).mean()))
    x = ((pts[:, 0] - c0[0]) * s0).astype(np.float32)
    y = ((pts[:, 1] - c0[1]) * s0).astype(np.float32)
    M = np.stack([x * x, x * y, x, y * y, y, np.ones_like(x)], -1).astype(
        np.float32)

    cpack = np.zeros((9, 36), np.float32)
    cpack[:9, C_I9H:C_I9H + 9] = 0.5 * np.eye(9, dtype=np.float32)
    pid = _pidx()
    for a in range(3):
        for b in range(3):
            cpack[pid[(a, b)], C_ET69 + 3 * a + b] = 1.0  # ET69[m, 3a+b]
    cpack[:3, C_I3:C_I3 + 3] = np.eye(3, dtype=np.float32)
    cpack[:9, C_V09] = 1.0 / 3.0
    cpack[:6, C_V06] = INV_SQRT3
    cpack[:3, C_SEL1:C_SEL1 + 3] = np.eye(3, dtype=np.float32)
    cpack[:3, C_SEL2 + 3:C_SEL2 + 6] = np.eye(3, dtype=np.float32)
    cpack[5, C_E5] = 1.0
    return M, cpack, float(c0[0]), float(c0[1]), float(s0)


def _tile128(a, ntiles):
    """[ntiles*128, F] -> [128, ntiles*F] with [p, t*F+f] = a[t*128+p, f]."""
    F = a.shape[1]
    return np.ascontiguousarray(
        a.reshape(ntiles, 128, F).transpose(1, 0, 2).reshape(128, ntiles * F)
    )


def build_nc(K):
    """Build the SPMD 8-core Bass program; returns compiled nc."""
    _, _, c0x, c0y, s0 = host_constants(K)
    inv_s0 = 1.0 / s0
    inv_s0sq = 1.0 / (s0 * s0)

    nc = bacc.Bacc("TRN2", target_bir_lowering=False, debug=False,
                   num_devices=NCORES)

    xt_d = nc.dram_tensor("xt", [128, CB * SH], F32, kind="ExternalInput")
    x_d = nc.dram_tensor("xin", [128, RT * N], F32, kind="ExternalInput")
    m1_d = nc.dram_tensor("m1s", [128, RT * 6], F32, kind="ExternalInput")
    m2_d = nc.dram_tensor("m2t", [128, CB * 6], F32, kind="ExternalInput")
    id_d = nc.dram_tensor("ident", [128, 128], F32, kind="ExternalInput")
    cp_d = nc.dram_tensor("cpack", [9, 36], F32, kind="ExternalInput")
    out_d = nc.dram_tensor("out", [3, 3], F32, kind="ExternalOutput")

    cp_in = nc.dram_tensor("cp_in", [128, CB * 3], F32)
    cp_out = nc.dram_tensor("cp_out", [NCORES * 128, CB * 3], F32,
                            addr_space="Shared")
    cr_in = nc.dram_tensor("cr_in", [6, 6], F32)
    cr_out = nc.dram_tensor("cr_out", [6, 6], F32, addr_space="Shared")
    stage = nc.dram_tensor("stage", [128], F32)
    mshuf = nc.dram_tensor("mshuf", [81], F32)

    groups = [list(range(NCORES))]

    with tile.TileContext(nc) as tc:
        with (
            tc.tile_pool(name="persist", bufs=1) as pp,
            tc.tile_pool(name="scratch", bufs=2) as sp,
            tc.tile_pool(name="ps_pt", bufs=2, space="PSUM") as ps,
            tc.tile_pool(name="ps_c", bufs=1, space="PSUM") as psc,
        ):
            # ---------- P0: loads (xt first - it gates the AllGather) ----
            XT = pp.tile([128, CB * SH], F32, tag="XT")
            XCH = 6 * SH
            for c in range(0, CB * SH, XCH):
                nc.sync.dma_start(XT[:, c:c + XCH], xt_d[:, c:c + XCH])
            cps = pp.tile([9, 36], F32, tag="cpk")
            nc.sync.dma_start(cps[:], cp_d[:])
            idn = pp.tile([128, 128], F32, tag="idn")
            nc.sync.dma_start(idn[:], id_d[:])
            m2t_s = pp.tile([128, CB * 6], F32, tag="m2")
            nc.sync.dma_start(m2t_s[:], m2_d[:])
            m1t_s = pp.tile([128, RT * 6], F32, tag="m1")
            nc.sync.dma_start(m1t_s[:], m1_d[:])
            X = pp.tile([128, RT * N], F32, tag="X")
            for t in range(RT):
                nc.sync.dma_start(X[:, t * N:(t + 1) * N],
                                  x_d[:, t * N:(t + 1) * N])

            # warm scalar-engine activation tables (Copy + Sqrt)
            wrm = sp.tile([1, 2], F32, tag="wrm")
            nc.vector.memset(wrm[:], 1.0)
            wrm2 = sp.tile([1, 2], F32, tag="wrm2")
            nc.scalar.activation(wrm2[:, 0:1], wrm[:, 0:1], AF.Copy)
            nc.scalar.activation(wrm2[:, 1:2], wrm[:, 1:2], AF.Sqrt)
            # all-ones tiles: PE-based partition broadcast/reduce (keeps the
            # Pool engine on its collectives ucode lib - lib swaps cost ~6.5us)
            ones9 = pp.tile([9, 9], F32, tag="ones9")
            nc.vector.memset(ones9[:], 1.0)
            onesr = pp.tile([1, 128], F32, tag="onesr")
            nc.vector.memset(onesr[:], 1.0)

            def XTj(j):
                return XT[:, j * SH:(j + 1) * SH]

            # ---------- P1: column top-3 partials -> AllGather ----------
            c8all = pp.tile([128, CB * 8], F32, tag="c8all")
            for j in range(CB):
                nc.vector.max(out=c8all[:, j * 8:j * 8 + 8], in_=XTj(j))
            c3all = pp.tile([128, CB * 3], F32, tag="c3all")
            nc.vector.tensor_copy(
                c3all[:].rearrange("p (j s) -> p j s", s=3),
                c8all[:].rearrange("p (j s) -> p j s", s=8)[:, :, 0:3])
            nc.scalar.dma_start(cp_in[:], c3all[:])
            nc.gpsimd.collective_compute(
                "AllGather", OP.bypass, replica_groups=groups,
                ins=[cp_in[:]], outs=[cp_out[:]])
            gath = pp.tile([128, NCORES * CB * 3], F32, tag="gath")
            nc.scalar.dma_start(
                gath[:].rearrange("p (k f) -> p k f", k=NCORES),
                cp_out[:].rearrange("(k p) f -> p k f", p=128))

            # ---------- P1b (during AG): row thresholds + row mask -------
            r8 = pp.tile([128, RT * 8], F32, tag="r8")
            for t in range(RT):
                nc.vector.max(out=r8[:, t * 8:t * 8 + 8],
                              in_=X[:, t * N:(t + 1) * N])
            trRow = pp.tile([1, SH], F32, tag="trRow")
            for t in range(RT):
                ptr = ps.tile([1, 128], F32, tag="tps")
                nc.tensor.transpose(ptr[:], r8[:, t * 8 + 2:t * 8 + 3], idn[:])
                nc.scalar.activation(trRow[:, t * 128:(t + 1) * 128], ptr[:],
                                     AF.Copy)
            trRow2 = pp.tile([1, SH], F32, tag="trRow2")
            nc.vector.tensor_scalar_max(trRow2[:], trRow[:], T0)
            # broadcast across partitions via PE ones-matmul
            trBp = psc.tile([128, SH], F32, tag="trBp")
            nc.tensor.matmul(trBp[:], onesr[:], trRow2[:], start=True,
                             stop=True)
            trB = pp.tile([128, SH], F32, tag="trB")
            nc.scalar.activation(trB[:], trBp[:], AF.Copy)

            # pre-mask: w = XT * (XT >= trB)   (in place, DVE only - Pool
            # tensor ops would force ucode lib swaps around the collectives)
            geb = pp.tile([128, CB * SH], F32, tag="geb")
            for j in range(CB):
                nc.vector.tensor_tensor(geb[:, j * SH:(j + 1) * SH], XTj(j),
                                        trB[:], OP.is_ge)
                nc.vector.tensor_tensor(XTj(j), XTj(j),
                                        geb[:, j * SH:(j + 1) * SH], OP.mult)

            # ---------- P2: post-AG combine + fused col mask + Gram ------
            # interleave per block so the PE can start on block 0 ASAP
            cm8 = pp.tile([128, CB * 8], F32, tag="cm8")
            gv = gath[:].rearrange("p (k j s) -> p j k s", k=NCORES, s=3)
            psB = psc.tile([6, SH], F32, tag="psB")
            for j in range(CB):
                nc.vector.max(out=cm8[:, j * 8:j * 8 + 8], in_=gv[:, j])
                nc.vector.scalar_tensor_tensor(XTj(j), XTj(j),
                                               cm8[:, j * 8 + 2:j * 8 + 3],
                                               XTj(j), OP.is_ge, OP.mult)
                nc.tensor.matmul(psB[:], m2t_s[:, j * 6:(j + 1) * 6], XTj(j),
                                 start=(j == 0), stop=(j == CB - 1))
            BtS = sp.tile([6, SH], F32, tag="BtS")
            nc.scalar.activation(BtS[:], psB[:], AF.Copy)
            Bcol = sp.tile([128, RT * 6], F32, tag="Bcol")
            for t in range(RT):
                pt = ps.tile([128, 6], F32, tag="tps")
                nc.tensor.transpose(pt[:], BtS[:, t * 128:(t + 1) * 128],
                                    idn[0:6, 0:6])
                nc.scalar.activation(Bcol[:, t * 6:(t + 1) * 6], pt[:], AF.Copy)
            psC = psc.tile([6, 6], F32, tag="psC")
            for t in range(RT):
                nc.tensor.matmul(psC[:], m1t_s[:, t * 6:(t + 1) * 6],
                                 Bcol[:, t * 6:(t + 1) * 6],
                                 start=(t == 0), stop=(t == RT - 1))
            Cp = sp.tile([6, 6], F32, tag="Cp")
            nc.scalar.activation(Cp[:], psC[:], AF.Copy)
            nc.sync.dma_start(cr_in[:], Cp[:])
            nc.gpsimd.collective_compute(
                "AllReduce", OP.add, replica_groups=groups,
                ins=[cr_in[:]], outs=[cr_out[:]])

            # ---------- tail ----------
            _tail(nc, pp, sp, ps, cps, idn, cr_out, stage, mshuf, out_d,
                  c0x, c0y, inv_s0, inv_s0sq, ones9)

    nc.compile()
    return nc


def _transpose(nc, ps, sp, in_sb, n, idn, tag):
    """PE-transpose square [n, n] SBUF -> new SBUF tile."""
    pt = ps.tile([n, n], F32, tag="tps")
    nc.tensor.transpose(pt[:], in_sb, idn[:n, :n])
    ot = sp.tile([n, n], F32, tag=f"ot_{tag}")
    nc.scalar.activation(ot[:], pt[:], AF.Copy)
    return ot


def _pow50(nc, ps, sp, m_sb, n, tag):
    """Direction of M^50 v via rescaled squarings M <- 2*(M@M);
    M50 = 2*((2*(M32@M16)) @ M2). All operands feed normalized eigvecs."""
    powers = {}
    cur = m_sb
    for i in range(1, 6):  # M2, M4, M8, M16, M32
        pm = ps.tile([n, n], F32, tag="tps")
        nc.tensor.matmul(pm[:], cur, cur, start=True, stop=True)
        nxt = sp.tile([n, n], F32, tag=f"pws_{tag}_{i}")
        nc.vector.tensor_scalar_mul(nxt[:], pm[:], 2.0)
        powers[2 ** i] = nxt
        cur = nxt[:]
    pm = ps.tile([n, n], F32, tag="tps")
    nc.tensor.matmul(pm[:], powers[32][:], powers[16][:], start=True, stop=True)
    m48 = sp.tile([n, n], F32, tag=f"pws_{tag}_48")
    nc.vector.tensor_scalar_mul(m48[:], pm[:], 2.0)
    pm = ps.tile([n, n], F32, tag="tps")
    nc.tensor.matmul(pm[:], m48[:], powers[2][:], start=True, stop=True)
    m50 = sp.tile([n, n], F32, tag=f"pws_{tag}_50")
    nc.vector.tensor_scalar_mul(m50[:], pm[:], 2.0)
    return m50


def _tail(nc, pp, sp, ps, cps, idn, cr_out, stage, mshuf, out_d,
          c0x, c0y, inv_s0, inv_s0sq, ones9):
    """C' (6x6 pre-centered Gram) -> Hartley -> L transforms -> Mmat ->
    power chains -> projection -> out."""
    e5 = cps[0:6, C_E5:C_E5 + 1]
    i9h = cps[0:9, C_I9H:C_I9H + 9]
    et69 = cps[0:6, C_ET69:C_ET69 + 9]
    i3c = cps[0:3, C_I3:C_I3 + 3]
    v09 = cps[0:9, C_V09:C_V09 + 1]
    v06 = cps[0:6, C_V06:C_V06 + 1]
    sel1 = cps[0:3, C_SEL1:C_SEL1 + 6]
    sel2 = cps[0:3, C_SEL2:C_SEL2 + 6]

    Cr = sp.tile([6, 6], F32, tag="Cr")
    nc.sync.dma_start(Cr[:], cr_out[:])
    CrT = _transpose(nc, ps, sp, Cr[:], 6, idn, "crt")

    sc = pp.tile([128, 96], F32, tag="tailsc")

    def scv(a, b):
        return sc[0:1, a:b]

    mo_ps = ps.tile([1, 6], F32, tag="tps")
    nc.tensor.matmul(mo_ps[:], e5, CrT[:], start=True, stop=True)
    nc.scalar.activation(scv(0, 6), mo_ps[:], AF.Copy)    # side1 moments
    mo_ps2 = ps.tile([1, 6], F32, tag="tps")
    nc.tensor.matmul(mo_ps2[:], e5, Cr[:], start=True, stop=True)
    nc.scalar.activation(scv(6, 12), mo_ps2[:], AF.Copy)  # side2 moments

    def pair(k):  # element k of each side: free idxs (k, k+6)
        return sc[0:1, 0:12].rearrange("p (g d) -> p d g", g=2)[:, k, :]

    # moments per side: [Sxx, Sxy, Sx, Syy, Sy, Sw]  (pre-centered coords)
    Sxx, Sx, Syy, Sy, Sw = pair(0), pair(2), pair(3), pair(4), pair(5)
    ws = scv(12, 14); nc.vector.tensor_scalar_add(ws, Sw, EPS)
    rws = scv(14, 16); nc.vector.reciprocal(rws, ws)
    cx = scv(16, 18); nc.vector.tensor_tensor(cx, Sx, rws, OP.mult)
    cy = scv(18, 20); nc.vector.tensor_tensor(cy, Sy, rws, OP.mult)
    t_a = scv(20, 22); nc.vector.tensor_tensor(t_a, cx, Sx, OP.mult)
    t_b = scv(22, 24); nc.vector.tensor_tensor(t_b, cy, Sy, OP.mult)
    cdS = scv(24, 26); nc.vector.tensor_tensor(cdS, t_a, t_b, OP.add)
    u_a = scv(26, 28); nc.vector.tensor_tensor(u_a, cx, cx, OP.mult)
    u_b = scv(28, 30); nc.vector.tensor_tensor(u_b, cy, cy, OP.mult)
    c2_ = scv(30, 32); nc.vector.tensor_tensor(c2_, u_a, u_b, OP.add)
    sq_ = scv(32, 34); nc.vector.tensor_tensor(sq_, Sxx, Syy, OP.add)
    n2c = scv(34, 36); nc.vector.tensor_scalar_mul(n2c, cdS, -2.0)
    c2w = scv(36, 38); nc.vector.tensor_tensor(c2w, c2_, Sw, OP.mult)
    m_ = scv(38, 40); nc.vector.tensor_tensor(m_, sq_, n2c, OP.add)
    m2_ = scv(40, 42); nc.vector.tensor_tensor(m2_, m_, c2w, OP.add)
    md2 = scv(42, 44); nc.vector.tensor_tensor(md2, m2_, rws, OP.mult)
    # md in ORIGINAL units: md2 * inv_s0^2 + EPS, then sqrt
    md2e = scv(44, 46)
    nc.vector.tensor_scalar(md2e, md2, inv_s0sq, EPS, OP.mult, OP.add)
    md = scv(46, 48); nc.scalar.activation(md, md2e, AF.Sqrt)
    mde = scv(48, 50); nc.vector.tensor_scalar_add(mde, md, EPS)
    rmd = scv(50, 52); nc.vector.reciprocal(rmd, mde)
    s_ = scv(52, 54); nc.vector.tensor_scalar_mul(s_, rmd, SQRT2)
    # hartley scale in pre-centered units; L values with signs folded in:
    # macx = -a*cx, na2cx = -a^2*cx, m2a2cx = -2a^2*cx, squares sign-free
    a_ = scv(54, 56); nc.vector.tensor_scalar_mul(a_, s_, inv_s0)
    na = scv(56, 58); nc.vector.tensor_scalar_mul(na, a_, -1.0)
    macx = scv(58, 60); nc.vector.tensor_tensor(macx, na, cx, OP.mult)
    macy = scv(60, 62); nc.vector.tensor_tensor(macy, na, cy, OP.mult)
    a2 = scv(62, 64); nc.vector.tensor_tensor(a2, a_, a_, OP.mult)
    na2cx = scv(64, 66); nc.vector.tensor_tensor(na2cx, a_, macx, OP.mult)
    na2cy = scv(66, 68); nc.vector.tensor_tensor(na2cy, a_, macy, OP.mult)
    m2a2cx = scv(68, 70); nc.vector.tensor_scalar_mul(m2a2cx, na2cx, 2.0)
    m2a2cy = scv(70, 72); nc.vector.tensor_scalar_mul(m2a2cy, na2cy, 2.0)
    a2cx2 = scv(72, 74); nc.vector.tensor_tensor(a2cx2, macx, macx, OP.mult)
    a2cy2 = scv(74, 76); nc.vector.tensor_tensor(a2cy2, macy, macy, OP.mult)
    a2cxcy = scv(76, 78); nc.vector.tensor_tensor(a2cxcy, macx, macy, OP.mult)
    # T entries (original units): cx_o = cx*inv_s0 + c0x ; nscx = -s*cx_o
    cxo = scv(78, 80)
    nc.vector.tensor_scalar(cxo, cx, inv_s0, c0x, OP.mult, OP.add)
    cyo = scv(80, 82)
    nc.vector.tensor_scalar(cyo, cy, inv_s0, c0y, OP.mult, OP.add)
    ns = scv(82, 84); nc.vector.tensor_scalar_mul(ns, s_, -1.0)
    nscx = scv(84, 86); nc.vector.tensor_tensor(nscx, ns, cxo, OP.mult)
    nscy = scv(86, 88); nc.vector.tensor_tensor(nscy, ns, cyo, OP.mult)

    # ----- staging buffer: [0:72] = L1^T|L2^T row-major, [72:90] = T1|T2 ---
    stg = pp.tile([1, 128], F32, tag="stg")
    nc.vector.memset(stg[0:1, 0:96], 0.0)
    Ls = stg[0:1, 0:72].rearrange("p (s v) -> p s v", s=2)   # [1, 2, 36]

    def lput(pos, val):
        nc.vector.tensor_copy(Ls[:, :, pos:pos + 1], val.unsqueeze(2))

    # Lbuf[6r+c] = L^T[r,c] = L[c,r]
    lput(0, a2); lput(7, a2); lput(21, a2)
    lput(12, m2a2cx); lput(13, na2cy); lput(14, a_)
    lput(25, na2cx); lput(27, m2a2cy); lput(28, a_)
    lput(30, a2cx2); lput(31, a2cxcy); lput(32, macx)
    lput(33, a2cy2); lput(34, macy)
    nc.vector.memset(Ls[:, :, 35:36], 1.0)
    # L-part staged first: the LT read gates the critical C2 chain; the
    # T-part (only needed after the 9x9 power chain) follows on another queue
    nc.sync.dma_start(stage[0:72], stg[0:1, 0:72])
    LT = sp.tile([6, 12], F32, tag="LT")   # [:, 0:6] = L1^T, [:, 6:12] = L2^T
    nc.sync.dma_start(
        LT[:].rearrange("r (s c) -> r s c", s=2),
        stage[0:72].rearrange("(s r c) -> r s c", s=2, c=6))

    Tv = stg[0:1, 72:90].rearrange("p (s v) -> p s v", s=2)  # [1, 2, 9]
    nc.vector.tensor_copy(Tv[:, :, 0:1], s_.unsqueeze(2))
    nc.vector.tensor_copy(Tv[:, :, 4:5], s_.unsqueeze(2))
    nc.vector.tensor_copy(Tv[:, :, 2:3], nscx.unsqueeze(2))
    nc.vector.tensor_copy(Tv[:, :, 5:6], nscy.unsqueeze(2))
    nc.vector.memset(Tv[:, :, 8:9], 1.0)
    nc.scalar.dma_start(stage[72:90], stg[0:1, 72:90])
    TT = sp.tile([3, 6], F32, tag="TT")    # [:, 0:3] = T1, [:, 3:6] = T2
    nc.scalar.dma_start(
        TT[:].rearrange("i (s j) -> i s j", s=2),
        stage[72:90].rearrange("(s i j) -> i s j", s=2, j=3))

    # ----- C2^T = L2 C'^T L1^T ; then G2 = E C2 E^T --------------------
    u2ps = ps.tile([6, 6], F32, tag="tps")
    nc.tensor.matmul(u2ps[:], LT[:, 6:12], CrT[:], start=True, stop=True)
    U2s = sp.tile([6, 6], F32, tag="U2s")
    nc.scalar.activation(U2s[:], u2ps[:], AF.Copy)
    U2T = _transpose(nc, ps, sp, U2s[:], 6, idn, "u2t")
    c2ps = ps.tile([6, 6], F32, tag="tps")
    nc.tensor.matmul(c2ps[:], U2T[:], LT[:, 0:6], start=True, stop=True)
    C2T = sp.tile([6, 6], F32, tag="C2T")
    nc.scalar.activation(C2T[:], c2ps[:], AF.Copy)

    z_ps = ps.tile([6, 9], F32, tag="tps")
    nc.tensor.matmul(z_ps[:], C2T[:], et69, start=True, stop=True)  # C2 E^T
    Zs = sp.tile([6, 9], F32, tag="Zs")
    nc.scalar.activation(Zs[:], z_ps[:], AF.Copy)
    g_ps = ps.tile([9, 9], F32, tag="tps")
    nc.tensor.matmul(g_ps[:], et69, Zs[:], start=True, stop=True)   # E @ Z
    G2 = sp.tile([9, 9], F32, tag="G2")
    nc.scalar.activation(G2[:], g_ps[:], AF.Copy)

    # Mmat[3p+q, 3r+s] = G2[3p+r, 3q+s]: bounce via DRAM, 3 parallel reads
    nc.sync.dma_start(mshuf[:], G2[:])
    Mmat = sp.tile([9, 9], F32, tag="Mmat")
    for p, eng in zip(range(3), (nc.sync, nc.scalar, nc.sync)):
        eng.dma_start(
            Mmat[3 * p:3 * p + 3, :].rearrange("q (r s) -> q r s", s=3),
            mshuf[:].rearrange("(p q1 r s) -> p q1 r s", p=3, q1=3, r=3)
            .transpose([0, 2, 1, 3])[p])

    # shifted scaled 9x9: Msp = Mmat/(2 lam) - I/2 (sign irrelevant, even pow)
    dg = sp.tile([9, 9], F32, tag="dg")
    nc.vector.tensor_tensor(dg[:], Mmat[:], i9h, OP.mult)  # diag/2
    lam2 = sp.tile([9, 1], F32, tag="lam2")
    nc.vector.tensor_reduce(lam2[:], dg[:], AX.X, OP.add)
    l2ps = ps.tile([9, 1], F32, tag="tps")
    nc.tensor.matmul(l2ps[:], ones9[:], lam2[:], start=True, stop=True)
    lam2r = sp.tile([9, 1], F32, tag="lam2r")
    nc.vector.tensor_copy(lam2r[:], l2ps[:])
    lam4 = sp.tile([9, 1], F32, tag="lam4")
    nc.vector.tensor_scalar_mul(lam4[:], lam2r[:], 4.0)  # = 2*lam
    inv2l = sp.tile([9, 1], F32, tag="inv2l")
    nc.vector.reciprocal(inv2l[:], lam4[:])
    Msp = sp.tile([9, 9], F32, tag="Msp")
    nc.vector.scalar_tensor_tensor(Msp[:], Mmat[:], inv2l[:], i9h,
                                   OP.mult, OP.subtract)
    M50 = _pow50(nc, ps, sp, Msp[:], 9, "m9")

    # w9 left UNNORMALIZED: E scales by ||w9||; all downstream eigvec math is
    # scale-free, only the final column scaling needs a 1/||w9|| fix, which
    # is computed here off the critical path and folded into f2 at the end.
    w9ps = ps.tile([1, 9], F32, tag="tps")
    nc.tensor.matmul(w9ps[:], v09, M50[:], start=True, stop=True)
    w9 = sp.tile([1, 9], F32, tag="w9")
    nc.vector.tensor_copy(w9[:], w9ps[:])
    w9sq = sp.tile([1, 9], F32, tag="w9sq")
    nc.vector.tensor_tensor(w9sq[:], w9[:], w9[:], OP.mult)
    nn9 = sp.tile([1, 1], F32, tag="nn9")
    nc.vector.tensor_reduce(nn9[:], w9sq[:], AX.X, OP.add)
    sr9 = sp.tile([1, 1], F32, tag="sr9")
    nc.scalar.activation(sr9[:], nn9[:], AF.Sqrt)
    rs9 = sp.tile([1, 1], F32, tag="rs9")
    nc.vector.reciprocal(rs9[:], sr9[:])
    r9ps = ps.tile([2, 1], F32, tag="tps")
    nc.tensor.matmul(r9ps[:], ones9[0:1, 0:2], rs9[:], start=True, stop=True)
    rs9b = sp.tile([2, 1], F32, tag="rs9b")
    nc.vector.tensor_copy(rs9b[:], r9ps[:])

    # E_raw^T via 3 tiny PE transposes (no DRAM bounce), then
    # E = T2^T (E_raw T1):  Y = mm(ETraw, T1) = E_raw T1 ; Es = mm(T2, Y)
    ETraw = sp.tile([3, 3], F32, tag="ETraw")
    for i in range(3):
        pt3 = ps.tile([3, 1], F32, tag="tps")
        nc.tensor.transpose(pt3[:], w9[0:1, 3 * i:3 * i + 3], idn[0:1, 0:1])
        nc.scalar.activation(ETraw[:, i:i + 1], pt3[:], AF.Copy)
    yps = ps.tile([3, 3], F32, tag="tps")
    nc.tensor.matmul(yps[:], ETraw[:], TT[:, 0:3], start=True, stop=True)
    Ys = sp.tile([3, 3], F32, tag="Ys")
    nc.vector.tensor_copy(Ys[:], yps[:])
    eps_ = ps.tile([3, 3], F32, tag="tps")
    nc.tensor.matmul(eps_[:], TT[:, 3:6], Ys[:], start=True, stop=True)
    Es = sp.tile([3, 3], F32, tag="Es")
    nc.vector.tensor_copy(Es[:], eps_[:])
    ETs = _transpose(nc, ps, sp, Es[:], 3, idn, "ets")

    # B = E^T E ; blockdiag 6x6 chain for v1 (max) and v3 (min)
    bps = ps.tile([3, 3], F32, tag="tps")
    nc.tensor.matmul(bps[:], Es[:], Es[:], start=True, stop=True)
    Bm = sp.tile([3, 3], F32, tag="Bm")
    nc.scalar.activation(Bm[:], bps[:], AF.Copy)
    dg3 = sp.tile([3, 3], F32, tag="dg3")
    nc.vector.tensor_tensor(dg3[:], Bm[:], i3c, OP.mult)
    lb = sp.tile([3, 1], F32, tag="lb")
    nc.vector.tensor_reduce(lb[:], dg3[:], AX.X, OP.add)
    lbps = ps.tile([3, 1], F32, tag="tps")
    nc.tensor.matmul(lbps[:], ones9[0:3, 0:3], lb[:], start=True, stop=True)
    lbr = sp.tile([3, 1], F32, tag="lbr")
    nc.vector.tensor_copy(lbr[:], lbps[:])
    invlb = sp.tile([3, 1], F32, tag="invlb")
    nc.vector.reciprocal(invlb[:], lbr[:])
    Bs3 = sp.tile([3, 3], F32, tag="Bs3")
    nc.vector.tensor_scalar_mul(Bs3[:], Bm[:], invlb[:])
    IB = sp.tile([3, 3], F32, tag="IB")
    nc.vector.tensor_tensor(IB[:], i3c, Bs3[:], OP.subtract)
    bdps = ps.tile([6, 6], F32, tag="tps")
    nc.tensor.matmul(bdps[:, 0:3], sel1, Bs3[:], start=True, stop=True)
    nc.tensor.matmul(bdps[:, 3:6], sel2, IB[:], start=True, stop=True)
    BD = sp.tile([6, 6], F32, tag="BD")
    nc.scalar.activation(BD[:], bdps[:], AF.Copy)
    BD50 = _pow50(nc, ps, sp, BD[:], 6, "m6")

    w6ps = ps.tile([1, 6], F32, tag="tps")
    nc.tensor.matmul(w6ps[:], v06, BD50[:], start=True, stop=True)
    w6 = sp.tile([1, 6], F32, tag="w6")
    nc.scalar.activation(w6[:], w6ps[:], AF.Copy)
    w6sq = sp.tile([1, 6], F32, tag="w6sq")
    nc.vector.tensor_tensor(w6sq[:], w6[:], w6[:], OP.mult)
    nn6 = sp.tile([1, 2], F32, tag="nn6")
    nc.vector.tensor_reduce(nn6[:].unsqueeze(2),
                            w6sq[:].rearrange("p (g d) -> p g d", g=2), AX.X,
                            OP.add)
    sr6 = sp.tile([1, 2], F32, tag="sr6")
    nc.scalar.activation(sr6[:], nn6[:], AF.Sqrt)
    rs6 = sp.tile([1, 2], F32, tag="rs6")
    nc.vector.reciprocal(rs6[:], sr6[:])
    vv = sp.tile([1, 6], F32, tag="vv")
    nc.vector.tensor_tensor(
        vv[:].rearrange("p (g d) -> p g d", g=2),
        w6[:].rearrange("p (g d) -> p g d", g=2),
        rs6[:].unsqueeze(2).to_broadcast([1, 2, 3]), OP.mult)

    # v2 = cross(v3, v1), normalized with EPS (as reference)
    aa = sp.tile([1, 6], F32, tag="aa")
    nc.vector.tensor_copy(
        aa[:].rearrange("p (r d) -> p r d", r=2),
        vv[:, 3:6].unsqueeze(1).to_broadcast([1, 2, 3]))
    bb = sp.tile([1, 6], F32, tag="bb")
    nc.vector.tensor_copy(
        bb[:].rearrange("p (r d) -> p r d", r=2),
        vv[:, 0:3].unsqueeze(1).to_broadcast([1, 2, 3]))
    cr1 = sp.tile([1, 3], F32, tag="cr1")
    nc.vector.tensor_tensor(cr1[:], aa[:, 1:4], bb[:, 2:5], OP.mult)
    cr2 = sp.tile([1, 3], F32, tag="cr2")
    nc.vector.tensor_tensor(cr2[:], aa[:, 2:5], bb[:, 1:4], OP.mult)
    v2r = sp.tile([1, 3], F32, tag="v2r")
    nc.vector.tensor_tensor(v2r[:], cr1[:], cr2[:], OP.subtract)
    v2sq = sp.tile([1, 3], F32, tag="v2sq")
    nc.vector.tensor_tensor(v2sq[:], v2r[:], v2r[:], OP.mult)
    nn2 = sp.tile([1, 1], F32, tag="nn2")
    nc.vector.tensor_reduce(nn2[:], v2sq[:], AX.X, OP.add)
    sr2 = sp.tile([1, 1], F32, tag="sr2")
    nc.scalar.activation(sr2[:], nn2[:], AF.Sqrt)
    sr2e = sp.tile([1, 1], F32, tag="sr2e")
    nc.vector.tensor_scalar_add(sr2e[:], sr2[:], EPS)
    rs2 = sp.tile([1, 1], F32, tag="rs2")
    nc.vector.reciprocal(rs2[:], sr2e[:])
    v2 = sp.tile([1, 3], F32, tag="v2")
    nc.vector.tensor_tensor(v2[:], v2r[:], rs2[:].to_broadcast([1, 3]), OP.mult)

    # V columns/rows via tiny PE transposes (no DRAM bounce)
    Vc = sp.tile([3, 2], F32, tag="Vc")
    ptv = ps.tile([3, 1], F32, tag="tps")
    nc.tensor.transpose(ptv[:], vv[0:1, 0:3], idn[0:1, 0:1])
    nc.scalar.activation(Vc[:, 0:1], ptv[:], AF.Copy)
    ptv2 = ps.tile([3, 1], F32, tag="tps")
    nc.tensor.transpose(ptv2[:], v2[0:1, 0:3], idn[0:1, 0:1])
    nc.scalar.activation(Vc[:, 1:2], ptv2[:], AF.Copy)
    ptvr = ps.tile([2, 3], F32, tag="tps")
    nc.tensor.transpose(ptvr[:], Vc[:], idn[0:3, 0:3])
    Vr = sp.tile([2, 3], F32, tag="Vr")
    nc.vector.tensor_copy(Vr[:], ptvr[:])
    evps = ps.tile([2, 3], F32, tag="tps")
    nc.tensor.matmul(evps[:], Vc[:], ETs[:], start=True, stop=True)
    Evr = sp.tile([2, 3], F32, tag="Evr")
    nc.scalar.activation(Evr[:], evps[:], AF.Copy)
    evsq = sp.tile([2, 3], F32, tag="evsq")
    nc.vector.tensor_tensor(evsq[:], Evr[:], Evr[:], OP.mult)
    ss2 = sp.tile([2, 1], F32, tag="ss2")
    nc.vector.tensor_reduce(ss2[:], evsq[:], AX.X, OP.add)
    sv = sp.tile([2, 1], F32, tag="sv")
    nc.scalar.activation(sv[:], ss2[:], AF.Sqrt)
    ssps = ps.tile([2, 1], F32, tag="tps")
    nc.tensor.matmul(ssps[:], ones9[0:2, 0:2], sv[:], start=True, stop=True)
    ssum = sp.tile([2, 1], F32, tag="ssum")
    nc.vector.tensor_copy(ssum[:], ssps[:])
    savg = sp.tile([2, 1], F32, tag="savg")
    nc.vector.tensor_scalar_mul(savg[:], ssum[:], 0.5)
    sve = sp.tile([2, 1], F32, tag="sve")
    nc.vector.tensor_scalar_add(sve[:], sv[:], EPS)
    rsv = sp.tile([2, 1], F32, tag="rsv")
    nc.vector.reciprocal(rsv[:], sve[:])
    f2 = sp.tile([2, 1], F32, tag="f2")
    nc.vector.tensor_tensor(f2[:], rsv[:], savg[:], OP.mult)
    f2n = sp.tile([2, 1], F32, tag="f2n")
    nc.vector.tensor_tensor(f2n[:], f2[:], rs9b[:], OP.mult)  # 1/||w9|| fix
    U2 = sp.tile([2, 3], F32, tag="U2")
    nc.vector.tensor_scalar_mul(U2[:], Evr[:], f2n[:])
    ops_ = ps.tile([3, 3], F32, tag="tps")
    nc.tensor.matmul(ops_[:], U2[:], Vr[:], start=True, stop=True)
    outs = sp.tile([3, 3], F32, tag="outs")
    nc.scalar.activation(outs[:], ops_[:], AF.Copy)
    nc.sync.dma_start(out_d[:], outs[:])


def make_in_maps(P, K):
    """Host-side shard + constant prep: list of 8 input dicts."""
    P = np.asarray(P, np.float32)
    K = np.asarray(K, np.float32)
    Pc = np.ascontiguousarray(P[:N, :N])
    M, cpack, _, _, _ = host_constants(K)
    m2t = _tile128(M, CB)
    ident = np.eye(128, dtype=np.float32)
    in_maps = []
    for k in range(NCORES):
        sh = Pc[k * SH:(k + 1) * SH]
        shT = np.ascontiguousarray(sh.T)          # [3072 cols, 384 rows]
        in_maps.append({
            "xt": _tile128(shT, CB),
            "xin": _tile128(sh, RT),
            "m1s": _tile128(M[k * SH:(k + 1) * SH], RT),
            "m2t": m2t,
            "ident": ident,
            "cpack": cpack,
        })
    return in_maps


_NC_CACHE = {}


def kernel(P, K):
    from concourse.bass_utils import run_bass_kernel_spmd
    key = (np.asarray(P).shape, np.asarray(K, np.float32).tobytes())
    if key not in _NC_CACHE:
        _NC_CACHE[key] = build_nc(K)
    nc = _NC_CACHE[key]
    in_maps = make_in_maps(P, K)
    res = run_bass_kernel_spmd(nc, in_maps, core_ids=list(range(NCORES)))
    return np.asarray(res.results[0]["out"], np.float32)


# revision 27
# speedup vs baseline: 1.1474x; 1.1474x over previous
"""Trainium2 Bass kernel for nn_EssentialMatrixEstimator.

Distribution: data-parallel over the N=3072 rows of Pc across 8 cores
(384 rows each).

v2 redesign vs the two-phase baseline:
  * The transposed shard W^T is shipped from host (pure layout prep), killing
    all 72 on-device PE transposes.
  * Monomials are PRE-CENTERED on host with fixed (c0, s0) derived from K
    (grid centroid / RMS), so a SINGLE well-conditioned 6x6 Gram C' suffices:
    the Hartley-centered Gram is recovered exactly as C2 = L1 C' L2^T where
    L1/L2 are 6x6 monomial shift/scale transforms built from C''s own
    moments (row/col 5). One AllGather (column top-3 partials) + one
    AllReduce (6x6 Gram) are the only collectives.
  * Gram computed as B^T = sum_j M2_j^T W^T_j (24 wide fp32 matmuls,
    stream 384) then C' = M1^T B (3 small matmuls after 3 PE transposes).
  * Masking fused: pre-AG  w = X * (X >= max(rowthr, T0)) (2 passes,
    split DVE/GpSimd, hidden under the AllGather); post-AG one fused
    scalar_tensor_tensor pass  w = (w >= colthr) * w  per 128-col block,
    pipelined into the Gram matmuls.
  * Power iterations: rescaled repeated squaring M^50 = 2*(2*(M32@M16)@M2)
    (M^48/M^64 were checked and are NOT converged enough - keep 50).
Validated against reference (rel err ~2e-5; tolerance 2e-2).
"""

import os

os.environ.setdefault("JAX_PLATFORMS", "axon")

import numpy as np

import concourse.bass as bass
import concourse.bass_isa as bass_isa
import concourse.mybir as mybir
import concourse.bacc as bacc
import concourse.tile as tile

NCORES = 8
N = 3072
SH = N // NCORES          # 384 rows per core
RT = SH // 128            # 3 row tiles per core
CB = N // 128             # 24 column blocks
F32 = mybir.dt.float32
AF = mybir.ActivationFunctionType
OP = mybir.AluOpType
AX = mybir.AxisListType

EPS = 1e-8
SQRT2 = 1.4142135623730951
INV_SQRT3 = 1.0 / 1.7320508075688772
T0 = float(np.nextafter(np.float32(0.01), np.float32(1)))  # x > 0.01 == x >= T0
H, W = 64, 64

DVE_BLOCKS = 15  # mask blocks on DVE; rest on gpsimd

# cpack const layout (tensor [9, 36]): column ranges
C_I9H = 0      # I9 * 0.5          [9, 9]
C_ET69 = 9     # E^T selector      [6, 9]
C_I3 = 18      # I3                [3, 3]
C_V09 = 21     # full(1/3)         [9, 1]
C_V06 = 22     # full(1/sqrt3)     [6, 1]
C_SEL1 = 23    # [I3 | 0]          [3, 6]
C_SEL2 = 29    # [0 | I3]          [3, 6]
C_E5 = 35      # e5 selector       [6, 1]

PAIRS = [(0, 0), (0, 1), (0, 2), (1, 1), (1, 2), (2, 2)]


def _pidx():
    d = {}
    for i, (a, b) in enumerate(PAIRS):
        d[(a, b)] = i
        d[(b, a)] = i
    return d


def host_constants(K):
    """Pre-centered monomial matrix + packed constants + (c0, s0)."""
    idx = np.arange(H * W, dtype=np.float32)
    pix = np.stack([idx % np.float32(W), np.floor(idx / np.float32(W))], -1)
    K_inv = np.linalg.inv(np.asarray(K, np.float32)).astype(np.float32)
    p1h = np.concatenate([pix[:N], np.ones((N, 1), np.float32)], -1)
    pts = (p1h @ K_inv.T)[:, :2].astype(np.float32)  # same grid both sides
    c0 = pts.mean(0).astype(np.float32)
    pc = pts - c0
    s0 = np.float32(SQRT2) / np.float32(np.sqrt((pc ** 2).sum(-1).mean()))
    x = ((pts[:, 0] - c0[0]) * s0).astype(np.float32)
    y = ((pts[:, 1] - c0[1]) * s0).astype(np.float32)
    M = np.stack([x * x, x * y, x, y * y, y, np.ones_like(x)], -1).astype(
        np.float32)

    cpack = np.zeros((9, 36), np.float32)
    cpack[:9, C_I9H:C_I9H + 9] = 0.5 * np.eye(9, dtype=np.float32)
    pid = _pidx()
    for a in range(3):
        for b in range(3):
            cpack[pid[(a, b)], C_ET69 + 3 * a + b] = 1.0  # ET69[m, 3a+b]
    cpack[:3, C_I3:C_I3 + 3] = np.eye(3, dtype=np.float32)
    cpack[:9, C_V09] = 1.0 / 3.0
    cpack[:6, C_V06] = INV_SQRT3
    cpack[:3, C_SEL1:C_SEL1 + 3] = np.eye(3, dtype=np.float32)
    cpack[:3, C_SEL2 + 3:C_SEL2 + 6] = np.eye(3, dtype=np.float32)
    cpack[5, C_E5] = 1.0
    return M, cpack, float(c0[0]), float(c0[1]), float(s0)


def _tile128(a, ntiles):
    """[ntiles*128, F] -> [128, ntiles*F] with [p, t*F+f] = a[t*128+p, f]."""
    F = a.shape[1]
    return np.ascontiguousarray(
        a.reshape(ntiles, 128, F).transpose(1, 0, 2).reshape(128, ntiles * F)
    )


def build_nc(K):
    """Build the SPMD 8-core Bass program; returns compiled nc."""
    _, _, c0x, c0y, s0 = host_constants(K)
    inv_s0 = 1.0 / s0
    inv_s0sq = 1.0 / (s0 * s0)

    nc = bacc.Bacc("TRN2", target_bir_lowering=False, debug=False,
                   num_devices=NCORES)

    xt_d = nc.dram_tensor("xt", [128, CB * SH], F32, kind="ExternalInput")
    x_d = nc.dram_tensor("xin", [128, RT * N], F32, kind="ExternalInput")
    m1_d = nc.dram_tensor("m1s", [128, RT * 6], F32, kind="ExternalInput")
    m2_d = nc.dram_tensor("m2t", [128, CB * 6], F32, kind="ExternalInput")
    id_d = nc.dram_tensor("ident", [128, 128], F32, kind="ExternalInput")
    cp_d = nc.dram_tensor("cpack", [9, 36], F32, kind="ExternalInput")
    out_d = nc.dram_tensor("out", [3, 3], F32, kind="ExternalOutput")

    cp_in = nc.dram_tensor("cp_in", [128, CB * 3], F32)
    cp_out = nc.dram_tensor("cp_out", [NCORES * 128, CB * 3], F32,
                            addr_space="Shared")
    cr_in = nc.dram_tensor("cr_in", [6, 6], F32)
    cr_out = nc.dram_tensor("cr_out", [6, 6], F32, addr_space="Shared")
    stage = nc.dram_tensor("stage", [128], F32)
    mshuf = nc.dram_tensor("mshuf", [81], F32)

    groups = [list(range(NCORES))]

    with tile.TileContext(nc) as tc:
        with (
            tc.tile_pool(name="persist", bufs=1) as pp,
            tc.tile_pool(name="scratch", bufs=2) as sp,
            tc.tile_pool(name="ps_pt", bufs=2, space="PSUM") as ps,
            tc.tile_pool(name="ps_c", bufs=1, space="PSUM") as psc,
        ):
            # ---------- P0: loads (xt first - it gates the AllGather; x
            # right behind - row thresholds + pre-mask must fit in the AG
            # window) ----
            XT = pp.tile([128, CB * SH], F32, tag="XT")
            XCH = 6 * SH
            for c in range(0, CB * SH, XCH):
                nc.sync.dma_start(XT[:, c:c + XCH], xt_d[:, c:c + XCH])
            X = pp.tile([128, RT * N], F32, tag="X")
            for t in range(RT):
                nc.sync.dma_start(X[:, t * N:(t + 1) * N],
                                  x_d[:, t * N:(t + 1) * N])
            cps = pp.tile([9, 36], F32, tag="cpk")
            nc.sync.dma_start(cps[:], cp_d[:])
            idn = pp.tile([128, 128], F32, tag="idn")
            nc.sync.dma_start(idn[:], id_d[:])
            m2t_s = pp.tile([128, CB * 6], F32, tag="m2")
            nc.sync.dma_start(m2t_s[:], m2_d[:])
            m1t_s = pp.tile([128, RT * 6], F32, tag="m1")
            nc.sync.dma_start(m1t_s[:], m1_d[:])

            # warm scalar-engine activation tables (Copy + Sqrt)
            wrm = sp.tile([1, 2], F32, tag="wrm")
            nc.vector.memset(wrm[:], 1.0)
            wrm2 = sp.tile([1, 2], F32, tag="wrm2")
            nc.scalar.activation(wrm2[:, 0:1], wrm[:, 0:1], AF.Copy)
            nc.scalar.activation(wrm2[:, 1:2], wrm[:, 1:2], AF.Sqrt)
            # all-ones tiles: PE-based partition broadcast/reduce (keeps the
            # Pool engine on its collectives ucode lib - lib swaps cost ~6.5us)
            ones9 = pp.tile([9, 9], F32, tag="ones9")
            nc.vector.memset(ones9[:], 1.0)
            onesr = pp.tile([1, 128], F32, tag="onesr")
            nc.vector.memset(onesr[:], 1.0)

            def XTj(j):
                return XT[:, j * SH:(j + 1) * SH]

            # ---------- P1: column top-3 partials -> AllGather ----------
            c8all = pp.tile([128, CB * 8], F32, tag="c8all")
            for j in range(CB):
                nc.vector.max(out=c8all[:, j * 8:j * 8 + 8], in_=XTj(j))
            c3all = pp.tile([128, CB * 3], F32, tag="c3all")
            nc.vector.tensor_copy(
                c3all[:].rearrange("p (j s) -> p j s", s=3),
                c8all[:].rearrange("p (j s) -> p j s", s=8)[:, :, 0:3])
            nc.scalar.dma_start(cp_in[:], c3all[:])
            nc.gpsimd.collective_compute(
                "AllGather", OP.bypass, replica_groups=groups,
                ins=[cp_in[:]], outs=[cp_out[:]])
            gath = pp.tile([128, NCORES * CB * 3], F32, tag="gath")
            nc.scalar.dma_start(
                gath[:].rearrange("p (k f) -> p k f", k=NCORES),
                cp_out[:].rearrange("(k p) f -> p k f", p=128))

            # ---------- P1b (during AG): row thresholds + row mask -------
            r8 = pp.tile([128, RT * 8], F32, tag="r8")
            for t in range(RT):
                nc.vector.max(out=r8[:, t * 8:t * 8 + 8],
                              in_=X[:, t * N:(t + 1) * N])
            trRow = pp.tile([1, SH], F32, tag="trRow")
            for t in range(RT):
                ptr = ps.tile([1, 128], F32, tag="tps")
                nc.tensor.transpose(ptr[:], r8[:, t * 8 + 2:t * 8 + 3], idn[:])
                nc.scalar.activation(trRow[:, t * 128:(t + 1) * 128], ptr[:],
                                     AF.Copy)
            trRow2 = pp.tile([1, SH], F32, tag="trRow2")
            nc.vector.tensor_scalar_max(trRow2[:], trRow[:], T0)
            # broadcast across partitions via PE ones-matmul
            trBp = psc.tile([128, SH], F32, tag="trBp")
            nc.tensor.matmul(trBp[:], onesr[:], trRow2[:], start=True,
                             stop=True)
            trB = pp.tile([128, SH], F32, tag="trB")
            nc.scalar.activation(trB[:], trBp[:], AF.Copy)

            # pre-mask: w = XT * (XT >= trB)   (in place, DVE only - Pool
            # tensor ops would force ucode lib swaps around the collectives).
            # 4 big chunks per pass: per-instruction overhead ~280ns, so few
            # wide ops beat 24 per-block ones.
            geb = pp.tile([128, CB * SH], F32, tag="geb")
            trBv = trB[:].unsqueeze(1).to_broadcast([128, 6, SH])
            MCH = 6 * SH
            for c in range(0, CB * SH, MCH):
                nc.vector.tensor_tensor(
                    geb[:, c:c + MCH].rearrange("p (b f) -> p b f", f=SH),
                    XT[:, c:c + MCH].rearrange("p (b f) -> p b f", f=SH),
                    trBv, OP.is_ge)
            for c in range(0, CB * SH, MCH):
                nc.vector.tensor_tensor(XT[:, c:c + MCH], XT[:, c:c + MCH],
                                        geb[:, c:c + MCH], OP.mult)

            # ---------- P2: post-AG combine + fused col mask + Gram ------
            # interleave per block so the PE can start on block 0 ASAP
            cm8 = pp.tile([128, CB * 8], F32, tag="cm8")
            gv = gath[:].rearrange("p (k j s) -> p j k s", k=NCORES, s=3)
            psB = psc.tile([6, SH], F32, tag="psB")
            for j in range(CB):
                nc.vector.max(out=cm8[:, j * 8:j * 8 + 8], in_=gv[:, j])
                nc.vector.scalar_tensor_tensor(XTj(j), XTj(j),
                                               cm8[:, j * 8 + 2:j * 8 + 3],
                                               XTj(j), OP.is_ge, OP.mult)
                nc.tensor.matmul(psB[:], m2t_s[:, j * 6:(j + 1) * 6], XTj(j),
                                 start=(j == 0), stop=(j == CB - 1))
            BtS = sp.tile([6, SH], F32, tag="BtS")
            nc.scalar.activation(BtS[:], psB[:], AF.Copy)
            Bcol = sp.tile([128, RT * 6], F32, tag="Bcol")
            for t in range(RT):
                pt = ps.tile([128, 6], F32, tag="tps")
                nc.tensor.transpose(pt[:], BtS[:, t * 128:(t + 1) * 128],
                                    idn[0:6, 0:6])
                nc.scalar.activation(Bcol[:, t * 6:(t + 1) * 6], pt[:], AF.Copy)
            psC = psc.tile([6, 6], F32, tag="psC")
            for t in range(RT):
                nc.tensor.matmul(psC[:], m1t_s[:, t * 6:(t + 1) * 6],
                                 Bcol[:, t * 6:(t + 1) * 6],
                                 start=(t == 0), stop=(t == RT - 1))
            Cp = sp.tile([6, 6], F32, tag="Cp")
            nc.scalar.activation(Cp[:], psC[:], AF.Copy)
            nc.sync.dma_start(cr_in[:], Cp[:])
            nc.gpsimd.collective_compute(
                "AllReduce", OP.add, replica_groups=groups,
                ins=[cr_in[:]], outs=[cr_out[:]])

            # ---------- tail ----------
            _tail(nc, pp, sp, ps, cps, idn, cr_out, stage, mshuf, out_d,
                  c0x, c0y, inv_s0, inv_s0sq, ones9)

    nc.compile()
    return nc


def _transpose(nc, ps, sp, in_sb, n, idn, tag):
    """PE-transpose square [n, n] SBUF -> new SBUF tile."""
    pt = ps.tile([n, n], F32, tag="tps")
    nc.tensor.transpose(pt[:], in_sb, idn[:n, :n])
    ot = sp.tile([n, n], F32, tag=f"ot_{tag}")
    nc.scalar.activation(ot[:], pt[:], AF.Copy)
    return ot


def _pow50(nc, ps, sp, m_sb, n, tag):
    """Direction of M^50 v via rescaled squarings M <- 2*(M@M);
    M50 = 2*((2*(M32@M16)) @ M2). All operands feed normalized eigvecs."""
    powers = {}
    cur = m_sb
    for i in range(1, 6):  # M2, M4, M8, M16, M32
        pm = ps.tile([n, n], F32, tag="tps")
        nc.tensor.matmul(pm[:], cur, cur, start=True, stop=True)
        nxt = sp.tile([n, n], F32, tag=f"pws_{tag}_{i}")
        nc.vector.tensor_scalar_mul(nxt[:], pm[:], 2.0)
        powers[2 ** i] = nxt
        cur = nxt[:]
    pm = ps.tile([n, n], F32, tag="tps")
    nc.tensor.matmul(pm[:], powers[32][:], powers[16][:], start=True, stop=True)
    m48 = sp.tile([n, n], F32, tag=f"pws_{tag}_48")
    nc.vector.tensor_scalar_mul(m48[:], pm[:], 2.0)
    pm = ps.tile([n, n], F32, tag="tps")
    nc.tensor.matmul(pm[:], m48[:], powers[2][:], start=True, stop=True)
    m50 = sp.tile([n, n], F32, tag=f"pws_{tag}_50")
    nc.vector.tensor_scalar_mul(m50[:], pm[:], 2.0)
    return m50


def _tail(nc, pp, sp, ps, cps, idn, cr_out, stage, mshuf, out_d,
          c0x, c0y, inv_s0, inv_s0sq, ones9):
    """C' (6x6 pre-centered Gram) -> Hartley -> L transforms -> Mmat ->
    power chains -> projection -> out."""
    e5 = cps[0:6, C_E5:C_E5 + 1]
    i9h = cps[0:9, C_I9H:C_I9H + 9]
    et69 = cps[0:6, C_ET69:C_ET69 + 9]
    i3c = cps[0:3, C_I3:C_I3 + 3]
    v09 = cps[0:9, C_V09:C_V09 + 1]
    v06 = cps[0:6, C_V06:C_V06 + 1]
    sel1 = cps[0:3, C_SEL1:C_SEL1 + 6]
    sel2 = cps[0:3, C_SEL2:C_SEL2 + 6]

    Cr = sp.tile([6, 6], F32, tag="Cr")
    nc.sync.dma_start(Cr[:], cr_out[:])
    CrT = _transpose(nc, ps, sp, Cr[:], 6, idn, "crt")

    sc = pp.tile([128, 96], F32, tag="tailsc")

    def scv(a, b):
        return sc[0:1, a:b]

    mo_ps = ps.tile([1, 6], F32, tag="tps")
    nc.tensor.matmul(mo_ps[:], e5, CrT[:], start=True, stop=True)
    nc.scalar.activation(scv(0, 6), mo_ps[:], AF.Copy)    # side1 moments
    mo_ps2 = ps.tile([1, 6], F32, tag="tps")
    nc.tensor.matmul(mo_ps2[:], e5, Cr[:], start=True, stop=True)
    nc.scalar.activation(scv(6, 12), mo_ps2[:], AF.Copy)  # side2 moments

    def pair(k):  # element k of each side: free idxs (k, k+6)
        return sc[0:1, 0:12].rearrange("p (g d) -> p d g", g=2)[:, k, :]

    # moments per side: [Sxx, Sxy, Sx, Syy, Sy, Sw]  (pre-centered coords)
    Sxx, Sx, Syy, Sy, Sw = pair(0), pair(2), pair(3), pair(4), pair(5)
    ws = scv(12, 14); nc.vector.tensor_scalar_add(ws, Sw, EPS)
    rws = scv(14, 16); nc.vector.reciprocal(rws, ws)
    cx = scv(16, 18); nc.vector.tensor_tensor(cx, Sx, rws, OP.mult)
    cy = scv(18, 20); nc.vector.tensor_tensor(cy, Sy, rws, OP.mult)
    t_a = scv(20, 22); nc.vector.tensor_tensor(t_a, cx, Sx, OP.mult)
    t_b = scv(22, 24); nc.vector.tensor_tensor(t_b, cy, Sy, OP.mult)
    cdS = scv(24, 26); nc.vector.tensor_tensor(cdS, t_a, t_b, OP.add)
    u_a = scv(26, 28); nc.vector.tensor_tensor(u_a, cx, cx, OP.mult)
    u_b = scv(28, 30); nc.vector.tensor_tensor(u_b, cy, cy, OP.mult)
    c2_ = scv(30, 32); nc.vector.tensor_tensor(c2_, u_a, u_b, OP.add)
    sq_ = scv(32, 34); nc.vector.tensor_tensor(sq_, Sxx, Syy, OP.add)
    n2c = scv(34, 36); nc.vector.tensor_scalar_mul(n2c, cdS, -2.0)
    c2w = scv(36, 38); nc.vector.tensor_tensor(c2w, c2_, Sw, OP.mult)
    m_ = scv(38, 40); nc.vector.tensor_tensor(m_, sq_, n2c, OP.add)
    m2_ = scv(40, 42); nc.vector.tensor_tensor(m2_, m_, c2w, OP.add)
    md2 = scv(42, 44); nc.vector.tensor_tensor(md2, m2_, rws, OP.mult)
    # md in ORIGINAL units: md2 * inv_s0^2 + EPS, then sqrt
    md2e = scv(44, 46)
    nc.vector.tensor_scalar(md2e, md2, inv_s0sq, EPS, OP.mult, OP.add)
    md = scv(46, 48); nc.scalar.activation(md, md2e, AF.Sqrt)
    mde = scv(48, 50); nc.vector.tensor_scalar_add(mde, md, EPS)
    rmd = scv(50, 52); nc.vector.reciprocal(rmd, mde)
    s_ = scv(52, 54); nc.vector.tensor_scalar_mul(s_, rmd, SQRT2)
    # hartley scale in pre-centered units; L values with signs folded in:
    # macx = -a*cx, na2cx = -a^2*cx, m2a2cx = -2a^2*cx, squares sign-free
    a_ = scv(54, 56); nc.vector.tensor_scalar_mul(a_, s_, inv_s0)
    na = scv(56, 58); nc.vector.tensor_scalar_mul(na, a_, -1.0)
    macx = scv(58, 60); nc.vector.tensor_tensor(macx, na, cx, OP.mult)
    macy = scv(60, 62); nc.vector.tensor_tensor(macy, na, cy, OP.mult)
    a2 = scv(62, 64); nc.vector.tensor_tensor(a2, a_, a_, OP.mult)
    na2cx = scv(64, 66); nc.vector.tensor_tensor(na2cx, a_, macx, OP.mult)
    na2cy = scv(66, 68); nc.vector.tensor_tensor(na2cy, a_, macy, OP.mult)
    m2a2cx = scv(68, 70); nc.vector.tensor_scalar_mul(m2a2cx, na2cx, 2.0)
    m2a2cy = scv(70, 72); nc.vector.tensor_scalar_mul(m2a2cy, na2cy, 2.0)
    a2cx2 = scv(72, 74); nc.vector.tensor_tensor(a2cx2, macx, macx, OP.mult)
    a2cy2 = scv(74, 76); nc.vector.tensor_tensor(a2cy2, macy, macy, OP.mult)
    a2cxcy = scv(76, 78); nc.vector.tensor_tensor(a2cxcy, macx, macy, OP.mult)
    # T entries (original units): cx_o = cx*inv_s0 + c0x ; nscx = -s*cx_o
    cxo = scv(78, 80)
    nc.vector.tensor_scalar(cxo, cx, inv_s0, c0x, OP.mult, OP.add)
    cyo = scv(80, 82)
    nc.vector.tensor_scalar(cyo, cy, inv_s0, c0y, OP.mult, OP.add)
    ns = scv(82, 84); nc.vector.tensor_scalar_mul(ns, s_, -1.0)
    nscx = scv(84, 86); nc.vector.tensor_tensor(nscx, ns, cxo, OP.mult)
    nscy = scv(86, 88); nc.vector.tensor_tensor(nscy, ns, cyo, OP.mult)

    # ----- staging buffer: [0:72] = L1^T|L2^T row-major, [72:90] = T1|T2 ---
    stg = pp.tile([1, 128], F32, tag="stg")
    nc.vector.memset(stg[0:1, 0:96], 0.0)
    Ls = stg[0:1, 0:72].rearrange("p (s v) -> p s v", s=2)   # [1, 2, 36]

    def lput(pos, val):
        nc.vector.tensor_copy(Ls[:, :, pos:pos + 1], val.unsqueeze(2))

    # Lbuf[6r+c] = L^T[r,c] = L[c,r]
    lput(0, a2); lput(7, a2); lput(21, a2)
    lput(12, m2a2cx); lput(13, na2cy); lput(14, a_)
    lput(25, na2cx); lput(27, m2a2cy); lput(28, a_)
    lput(30, a2cx2); lput(31, a2cxcy); lput(32, macx)
    lput(33, a2cy2); lput(34, macy)
    nc.vector.memset(Ls[:, :, 35:36], 1.0)
    # L-part staged first: the LT read gates the critical C2 chain; the
    # T-part (only needed after the 9x9 power chain) follows on another queue
    nc.sync.dma_start(stage[0:72], stg[0:1, 0:72])
    LT = sp.tile([6, 12], F32, tag="LT")   # [:, 0:6] = L1^T, [:, 6:12] = L2^T
    nc.sync.dma_start(
        LT[:].rearrange("r (s c) -> r s c", s=2),
        stage[0:72].rearrange("(s r c) -> r s c", s=2, c=6))

    Tv = stg[0:1, 72:90].rearrange("p (s v) -> p s v", s=2)  # [1, 2, 9]
    nc.vector.tensor_copy(Tv[:, :, 0:1], s_.unsqueeze(2))
    nc.vector.tensor_copy(Tv[:, :, 4:5], s_.unsqueeze(2))
    nc.vector.tensor_copy(Tv[:, :, 2:3], nscx.unsqueeze(2))
    nc.vector.tensor_copy(Tv[:, :, 5:6], nscy.unsqueeze(2))
    nc.vector.memset(Tv[:, :, 8:9], 1.0)
    nc.scalar.dma_start(stage[72:90], stg[0:1, 72:90])
    TT = sp.tile([3, 6], F32, tag="TT")    # [:, 0:3] = T1, [:, 3:6] = T2
    nc.scalar.dma_start(
        TT[:].rearrange("i (s j) -> i s j", s=2),
        stage[72:90].rearrange("(s i j) -> i s j", s=2, j=3))

    # ----- C2^T = L2 C'^T L1^T ; then G2 = E C2 E^T --------------------
    u2ps = ps.tile([6, 6], F32, tag="tps")
    nc.tensor.matmul(u2ps[:], LT[:, 6:12], CrT[:], start=True, stop=True)
    U2s = sp.tile([6, 6], F32, tag="U2s")
    nc.scalar.activation(U2s[:], u2ps[:], AF.Copy)
    U2T = _transpose(nc, ps, sp, U2s[:], 6, idn, "u2t")
    c2ps = ps.tile([6, 6], F32, tag="tps")
    nc.tensor.matmul(c2ps[:], U2T[:], LT[:, 0:6], start=True, stop=True)
    C2T = sp.tile([6, 6], F32, tag="C2T")
    nc.scalar.activation(C2T[:], c2ps[:], AF.Copy)

    z_ps = ps.tile([6, 9], F32, tag="tps")
    nc.tensor.matmul(z_ps[:], C2T[:], et69, start=True, stop=True)  # C2 E^T
    Zs = sp.tile([6, 9], F32, tag="Zs")
    nc.scalar.activation(Zs[:], z_ps[:], AF.Copy)
    g_ps = ps.tile([9, 9], F32, tag="tps")
    nc.tensor.matmul(g_ps[:], et69, Zs[:], start=True, stop=True)   # E @ Z
    G2 = sp.tile([9, 9], F32, tag="G2")
    nc.scalar.activation(G2[:], g_ps[:], AF.Copy)

    # Mmat[3p+q, 3r+s] = G2[3p+r, 3q+s]: bounce via DRAM, 3 parallel reads
    nc.sync.dma_start(mshuf[:], G2[:])
    Mmat = sp.tile([9, 9], F32, tag="Mmat")
    for p, eng in zip(range(3), (nc.sync, nc.scalar, nc.sync)):
        eng.dma_start(
            Mmat[3 * p:3 * p + 3, :].rearrange("q (r s) -> q r s", s=3),
            mshuf[:].rearrange("(p q1 r s) -> p q1 r s", p=3, q1=3, r=3)
            .transpose([0, 2, 1, 3])[p])

    # shifted scaled 9x9: Msp = Mmat/(2 lam) - I/2 (sign irrelevant, even pow)
    dg = sp.tile([9, 9], F32, tag="dg")
    nc.vector.tensor_tensor(dg[:], Mmat[:], i9h, OP.mult)  # diag/2
    lam2 = sp.tile([9, 1], F32, tag="lam2")
    nc.vector.tensor_reduce(lam2[:], dg[:], AX.X, OP.add)
    l2ps = ps.tile([9, 1], F32, tag="tps")
    nc.tensor.matmul(l2ps[:], ones9[:], lam2[:], start=True, stop=True)
    lam2r = sp.tile([9, 1], F32, tag="lam2r")
    nc.vector.tensor_copy(lam2r[:], l2ps[:])
    lam4 = sp.tile([9, 1], F32, tag="lam4")
    nc.vector.tensor_scalar_mul(lam4[:], lam2r[:], 4.0)  # = 2*lam
    inv2l = sp.tile([9, 1], F32, tag="inv2l")
    nc.vector.reciprocal(inv2l[:], lam4[:])
    Msp = sp.tile([9, 9], F32, tag="Msp")
    nc.vector.scalar_tensor_tensor(Msp[:], Mmat[:], inv2l[:], i9h,
                                   OP.mult, OP.subtract)
    M50 = _pow50(nc, ps, sp, Msp[:], 9, "m9")

    # w9 left UNNORMALIZED: E scales by ||w9||; all downstream eigvec math is
    # scale-free, only the final column scaling needs a 1/||w9|| fix, which
    # is computed here off the critical path and folded into f2 at the end.
    w9ps = ps.tile([1, 9], F32, tag="tps")
    nc.tensor.matmul(w9ps[:], v09, M50[:], start=True, stop=True)
    w9 = sp.tile([1, 9], F32, tag="w9")
    nc.vector.tensor_copy(w9[:], w9ps[:])
    w9sq = sp.tile([1, 9], F32, tag="w9sq")
    nc.vector.tensor_tensor(w9sq[:], w9[:], w9[:], OP.mult)
    nn9 = sp.tile([1, 1], F32, tag="nn9")
    nc.vector.tensor_reduce(nn9[:], w9sq[:], AX.X, OP.add)
    sr9 = sp.tile([1, 1], F32, tag="sr9")
    nc.scalar.activation(sr9[:], nn9[:], AF.Sqrt)
    rs9 = sp.tile([1, 1], F32, tag="rs9")
    nc.vector.reciprocal(rs9[:], sr9[:])
    r9ps = ps.tile([2, 1], F32, tag="tps")
    nc.tensor.matmul(r9ps[:], ones9[0:1, 0:2], rs9[:], start=True, stop=True)
    rs9b = sp.tile([2, 1], F32, tag="rs9b")
    nc.vector.tensor_copy(rs9b[:], r9ps[:])

    # E_raw^T via 3 tiny PE transposes (no DRAM bounce), then
    # E = T2^T (E_raw T1):  Y = mm(ETraw, T1) = E_raw T1 ; Es = mm(T2, Y)
    ETraw = sp.tile([3, 3], F32, tag="ETraw")
    for i in range(3):
        pt3 = ps.tile([3, 1], F32, tag="tps")
        nc.tensor.transpose(pt3[:], w9[0:1, 3 * i:3 * i + 3], idn[0:1, 0:1])
        nc.scalar.activation(ETraw[:, i:i + 1], pt3[:], AF.Copy)
    yps = ps.tile([3, 3], F32, tag="tps")
    nc.tensor.matmul(yps[:], ETraw[:], TT[:, 0:3], start=True, stop=True)
    Ys = sp.tile([3, 3], F32, tag="Ys")
    nc.vector.tensor_copy(Ys[:], yps[:])
    eps_ = ps.tile([3, 3], F32, tag="tps")
    nc.tensor.matmul(eps_[:], TT[:, 3:6], Ys[:], start=True, stop=True)
    Es = sp.tile([3, 3], F32, tag="Es")
    nc.vector.tensor_copy(Es[:], eps_[:])
    ETs = _transpose(nc, ps, sp, Es[:], 3, idn, "ets")

    # B = E^T E ; blockdiag 6x6 chain for v1 (max) and v3 (min)
    bps = ps.tile([3, 3], F32, tag="tps")
    nc.tensor.matmul(bps[:], Es[:], Es[:], start=True, stop=True)
    Bm = sp.tile([3, 3], F32, tag="Bm")
    nc.scalar.activation(Bm[:], bps[:], AF.Copy)
    dg3 = sp.tile([3, 3], F32, tag="dg3")
    nc.vector.tensor_tensor(dg3[:], Bm[:], i3c, OP.mult)
    lb = sp.tile([3, 1], F32, tag="lb")
    nc.vector.tensor_reduce(lb[:], dg3[:], AX.X, OP.add)
    lbps = ps.tile([3, 1], F32, tag="tps")
    nc.tensor.matmul(lbps[:], ones9[0:3, 0:3], lb[:], start=True, stop=True)
    lbr = sp.tile([3, 1], F32, tag="lbr")
    nc.vector.tensor_copy(lbr[:], lbps[:])
    invlb = sp.tile([3, 1], F32, tag="invlb")
    nc.vector.reciprocal(invlb[:], lbr[:])
    Bs3 = sp.tile([3, 3], F32, tag="Bs3")
    nc.vector.tensor_scalar_mul(Bs3[:], Bm[:], invlb[:])
    IB = sp.tile([3, 3], F32, tag="IB")
    nc.vector.tensor_tensor(IB[:], i3c, Bs3[:], OP.subtract)
    bdps = ps.tile([6, 6], F32, tag="tps")
    nc.tensor.matmul(bdps[:, 0:3], sel1, Bs3[:], start=True, stop=True)
    nc.tensor.matmul(bdps[:, 3:6], sel2, IB[:], start=True, stop=True)
    BD = sp.tile([6, 6], F32, tag="BD")
    nc.scalar.activation(BD[:], bdps[:], AF.Copy)
    BD50 = _pow50(nc, ps, sp, BD[:], 6, "m6")

    w6ps = ps.tile([1, 6], F32, tag="tps")
    nc.tensor.matmul(w6ps[:], v06, BD50[:], start=True, stop=True)
    w6 = sp.tile([1, 6], F32, tag="w6")
    nc.scalar.activation(w6[:], w6ps[:], AF.Copy)
    w6sq = sp.tile([1, 6], F32, tag="w6sq")
    nc.vector.tensor_tensor(w6sq[:], w6[:], w6[:], OP.mult)
    nn6 = sp.tile([1, 2], F32, tag="nn6")
    nc.vector.tensor_reduce(nn6[:].unsqueeze(2),
                            w6sq[:].rearrange("p (g d) -> p g d", g=2), AX.X,
                            OP.add)
    sr6 = sp.tile([1, 2], F32, tag="sr6")
    nc.scalar.activation(sr6[:], nn6[:], AF.Sqrt)
    rs6 = sp.tile([1, 2], F32, tag="rs6")
    nc.vector.reciprocal(rs6[:], sr6[:])
    vv = sp.tile([1, 6], F32, tag="vv")
    nc.vector.tensor_tensor(
        vv[:].rearrange("p (g d) -> p g d", g=2),
        w6[:].rearrange("p (g d) -> p g d", g=2),
        rs6[:].unsqueeze(2).to_broadcast([1, 2, 3]), OP.mult)

    # v2 = cross(v3, v1), normalized with EPS (as reference)
    aa = sp.tile([1, 6], F32, tag="aa")
    nc.vector.tensor_copy(
        aa[:].rearrange("p (r d) -> p r d", r=2),
        vv[:, 3:6].unsqueeze(1).to_broadcast([1, 2, 3]))
    bb = sp.tile([1, 6], F32, tag="bb")
    nc.vector.tensor_copy(
        bb[:].rearrange("p (r d) -> p r d", r=2),
        vv[:, 0:3].unsqueeze(1).to_broadcast([1, 2, 3]))
    cr1 = sp.tile([1, 3], F32, tag="cr1")
    nc.vector.tensor_tensor(cr1[:], aa[:, 1:4], bb[:, 2:5], OP.mult)
    cr2 = sp.tile([1, 3], F32, tag="cr2")
    nc.vector.tensor_tensor(cr2[:], aa[:, 2:5], bb[:, 1:4], OP.mult)
    v2r = sp.tile([1, 3], F32, tag="v2r")
    nc.vector.tensor_tensor(v2r[:], cr1[:], cr2[:], OP.subtract)
    v2sq = sp.tile([1, 3], F32, tag="v2sq")
    nc.vector.tensor_tensor(v2sq[:], v2r[:], v2r[:], OP.mult)
    nn2 = sp.tile([1, 1], F32, tag="nn2")
    nc.vector.tensor_reduce(nn2[:], v2sq[:], AX.X, OP.add)
    sr2 = sp.tile([1, 1], F32, tag="sr2")
    nc.scalar.activation(sr2[:], nn2[:], AF.Sqrt)
    sr2e = sp.tile([1, 1], F32, tag="sr2e")
    nc.vector.tensor_scalar_add(sr2e[:], sr2[:], EPS)
    rs2 = sp.tile([1, 1], F32, tag="rs2")
    nc.vector.reciprocal(rs2[:], sr2e[:])
    v2 = sp.tile([1, 3], F32, tag="v2")
    nc.vector.tensor_tensor(v2[:], v2r[:], rs2[:].to_broadcast([1, 3]), OP.mult)

    # V columns/rows via tiny PE transposes (no DRAM bounce)
    Vc = sp.tile([3, 2], F32, tag="Vc")
    ptv = ps.tile([3, 1], F32, tag="tps")
    nc.tensor.transpose(ptv[:], vv[0:1, 0:3], idn[0:1, 0:1])
    nc.scalar.activation(Vc[:, 0:1], ptv[:], AF.Copy)
    ptv2 = ps.tile([3, 1], F32, tag="tps")
    nc.tensor.transpose(ptv2[:], v2[0:1, 0:3], idn[0:1, 0:1])
    nc.scalar.activation(Vc[:, 1:2], ptv2[:], AF.Copy)
    ptvr = ps.tile([2, 3], F32, tag="tps")
    nc.tensor.transpose(ptvr[:], Vc[:], idn[0:3, 0:3])
    Vr = sp.tile([2, 3], F32, tag="Vr")
    nc.vector.tensor_copy(Vr[:], ptvr[:])
    evps = ps.tile([2, 3], F32, tag="tps")
    nc.tensor.matmul(evps[:], Vc[:], ETs[:], start=True, stop=True)
    Evr = sp.tile([2, 3], F32, tag="Evr")
    nc.scalar.activation(Evr[:], evps[:], AF.Copy)
    evsq = sp.tile([2, 3], F32, tag="evsq")
    nc.vector.tensor_tensor(evsq[:], Evr[:], Evr[:], OP.mult)
    ss2 = sp.tile([2, 1], F32, tag="ss2")
    nc.vector.tensor_reduce(ss2[:], evsq[:], AX.X, OP.add)
    sv = sp.tile([2, 1], F32, tag="sv")
    nc.scalar.activation(sv[:], ss2[:], AF.Sqrt)
    ssps = ps.tile([2, 1], F32, tag="tps")
    nc.tensor.matmul(ssps[:], ones9[0:2, 0:2], sv[:], start=True, stop=True)
    ssum = sp.tile([2, 1], F32, tag="ssum")
    nc.vector.tensor_copy(ssum[:], ssps[:])
    savg = sp.tile([2, 1], F32, tag="savg")
    nc.vector.tensor_scalar_mul(savg[:], ssum[:], 0.5)
    sve = sp.tile([2, 1], F32, tag="sve")
    nc.vector.tensor_scalar_add(sve[:], sv[:], EPS)
    rsv = sp.tile([2, 1], F32, tag="rsv")
    nc.vector.reciprocal(rsv[:], sve[:])
    f2 = sp.tile([2, 1], F32, tag="f2")
    nc.vector.tensor_tensor(f2[:], rsv[:], savg[:], OP.mult)
    f2n = sp.tile([2, 1], F32, tag="f2n")
    nc.vector.tensor_tensor(f2n[:], f2[:], rs9b[:], OP.mult)  # 1/||w9|| fix
    U2 = sp.tile([2, 3], F32, tag="U2")
    nc.vector.tensor_scalar_mul(U2[:], Evr[:], f2n[:])
    ops_ = ps.tile([3, 3], F32, tag="tps")
    nc.tensor.matmul(ops_[:], U2[:], Vr[:], start=True, stop=True)
    outs = sp.tile([3, 3], F32, tag="outs")
    nc.scalar.activation(outs[:], ops_[:], AF.Copy)
    nc.sync.dma_start(out_d[:], outs[:])


def make_in_maps(P, K):
    """Host-side shard + constant prep: list of 8 input dicts."""
    P = np.asarray(P, np.float32)
    K = np.asarray(K, np.float32)
    Pc = np.ascontiguousarray(P[:N, :N])
    M, cpack, _, _, _ = host_constants(K)
    m2t = _tile128(M, CB)
    ident = np.eye(128, dtype=np.float32)
    in_maps = []
    for k in range(NCORES):
        sh = Pc[k * SH:(k + 1) * SH]
        shT = np.ascontiguousarray(sh.T)          # [3072 cols, 384 rows]
        in_maps.append({
            "xt": _tile128(shT, CB),
            "xin": _tile128(sh, RT),
            "m1s": _tile128(M[k * SH:(k + 1) * SH], RT),
            "m2t": m2t,
            "ident": ident,
            "cpack": cpack,
        })
    return in_maps


_NC_CACHE = {}


def kernel(P, K):
    from concourse.bass_utils import run_bass_kernel_spmd
    key = (np.asarray(P).shape, np.asarray(K, np.float32).tobytes())
    if key not in _NC_CACHE:
        _NC_CACHE[key] = build_nc(K)
    nc = _NC_CACHE[key]
    in_maps = make_in_maps(P, K)
    res = run_bass_kernel_spmd(nc, in_maps, core_ids=list(range(NCORES)))
    return np.asarray(res.results[0]["out"], np.float32)


# revision 32
# speedup vs baseline: 1.1538x; 1.0056x over previous
"""Trainium2 Bass kernel for nn_EssentialMatrixEstimator.

Distribution: data-parallel over the N=3072 rows of Pc across 8 cores
(384 rows each).

v2 redesign vs the two-phase baseline:
  * The transposed shard W^T is shipped from host (pure layout prep), killing
    all 72 on-device PE transposes.
  * Monomials are PRE-CENTERED on host with fixed (c0, s0) derived from K
    (grid centroid / RMS), so a SINGLE well-conditioned 6x6 Gram C' suffices:
    the Hartley-centered Gram is recovered exactly as C2 = L1 C' L2^T where
    L1/L2 are 6x6 monomial shift/scale transforms built from C''s own
    moments (row/col 5). One AllGather (column top-3 partials) + one
    AllReduce (6x6 Gram) are the only collectives.
  * Gram computed as B^T = sum_j M2_j^T W^T_j (24 wide fp32 matmuls,
    stream 384) then C' = M1^T B (3 small matmuls after 3 PE transposes).
  * Masking fused: pre-AG  w = X * (X >= max(rowthr, T0)) (2 passes,
    split DVE/GpSimd, hidden under the AllGather); post-AG one fused
    scalar_tensor_tensor pass  w = (w >= colthr) * w  per 128-col block,
    pipelined into the Gram matmuls.
  * Power iterations: rescaled repeated squaring M^50 = 2*(2*(M32@M16)@M2)
    (M^48/M^64 were checked and are NOT converged enough - keep 50).
Validated against reference (rel err ~2e-5; tolerance 2e-2).
"""

import os

os.environ.setdefault("JAX_PLATFORMS", "axon")

import numpy as np

import concourse.bass as bass
import concourse.bass_isa as bass_isa
import concourse.mybir as mybir
import concourse.bacc as bacc
import concourse.tile as tile

NCORES = 8
N = 3072
SH = N // NCORES          # 384 rows per core
RT = SH // 128            # 3 row tiles per core
CB = N // 128             # 24 column blocks
F32 = mybir.dt.float32
AF = mybir.ActivationFunctionType
OP = mybir.AluOpType
AX = mybir.AxisListType

EPS = 1e-8
SQRT2 = 1.4142135623730951
INV_SQRT3 = 1.0 / 1.7320508075688772
T0 = float(np.nextafter(np.float32(0.01), np.float32(1)))  # x > 0.01 == x >= T0
H, W = 64, 64

DVE_BLOCKS = 15  # mask blocks on DVE; rest on gpsimd

# cpack const layout (tensor [9, 120]): column ranges
C_I9H = 0      # I9 * 0.5          [9, 9]
C_ET69 = 9     # E^T selector      [6, 9]
C_I3 = 18      # I3                [3, 3]
C_V09 = 21     # full(1/3)         [9, 1]
C_V06 = 22     # full(1/sqrt3)     [6, 1]
C_SEL1 = 23    # [I3 | 0]          [3, 6]
C_SEL2 = 29    # [0 | I3]          [3, 6]
C_E5 = 35      # e5 selector       [6, 1]
C_LM = 36      # 12 L^T position masks [6, 6] each    [6, 72]
C_TM = 108     # 4 T position masks [3, 3] each       [3, 12]
CPW = 120

# L^T position masks: (value key -> list of (row, col) in L^T); the scv
# column of each value (pair base offset) is recorded alongside.
LMASKS = [
    ("a2", 62, [(0, 0), (1, 1), (3, 3)]),
    ("a", 54, [(2, 2), (4, 4)]),
    ("m2a2cx", 68, [(2, 0)]),
    ("na2cy", 66, [(2, 1)]),
    ("na2cx", 64, [(4, 1)]),
    ("m2a2cy", 70, [(4, 3)]),
    ("a2cx2", 72, [(5, 0)]),
    ("a2cxcy", 76, [(5, 1)]),
    ("macx", 58, [(5, 2)]),
    ("a2cy2", 74, [(5, 3)]),
    ("macy", 60, [(5, 4)]),
    ("one", None, [(5, 5)]),
]
TMASKS = [
    ("s", 52, [(0, 0), (1, 1)]),
    ("nscx", 84, [(0, 2)]),
    ("nscy", 86, [(1, 2)]),
    ("one", None, [(2, 2)]),
]

PAIRS = [(0, 0), (0, 1), (0, 2), (1, 1), (1, 2), (2, 2)]


def _pidx():
    d = {}
    for i, (a, b) in enumerate(PAIRS):
        d[(a, b)] = i
        d[(b, a)] = i
    return d


def host_constants(K):
    """Pre-centered monomial matrix + packed constants + (c0, s0)."""
    idx = np.arange(H * W, dtype=np.float32)
    pix = np.stack([idx % np.float32(W), np.floor(idx / np.float32(W))], -1)
    K_inv = np.linalg.inv(np.asarray(K, np.float32)).astype(np.float32)
    p1h = np.concatenate([pix[:N], np.ones((N, 1), np.float32)], -1)
    pts = (p1h @ K_inv.T)[:, :2].astype(np.float32)  # same grid both sides
    c0 = pts.mean(0).astype(np.float32)
    pc = pts - c0
    s0 = np.float32(SQRT2) / np.float32(np.sqrt((pc ** 2).sum(-1).mean()))
    x = ((pts[:, 0] - c0[0]) * s0).astype(np.float32)
    y = ((pts[:, 1] - c0[1]) * s0).astype(np.float32)
    M = np.stack([x * x, x * y, x, y * y, y, np.ones_like(x)], -1).astype(
        np.float32)

    cpack = np.zeros((9, CPW), np.float32)
    cpack[:9, C_I9H:C_I9H + 9] = 0.5 * np.eye(9, dtype=np.float32)
    pid = _pidx()
    for a in range(3):
        for b in range(3):
            cpack[pid[(a, b)], C_ET69 + 3 * a + b] = 1.0  # ET69[m, 3a+b]
    cpack[:3, C_I3:C_I3 + 3] = np.eye(3, dtype=np.float32)
    cpack[:9, C_V09] = 1.0 / 3.0
    cpack[:6, C_V06] = INV_SQRT3
    cpack[:3, C_SEL1:C_SEL1 + 3] = np.eye(3, dtype=np.float32)
    cpack[:3, C_SEL2 + 3:C_SEL2 + 6] = np.eye(3, dtype=np.float32)
    cpack[5, C_E5] = 1.0
    for k, (_, _, poss) in enumerate(LMASKS):
        for (r, c) in poss:
            cpack[r, C_LM + 6 * k + c] = 1.0
    for t, (_, _, poss) in enumerate(TMASKS):
        for (r, c) in poss:
            cpack[r, C_TM + 3 * t + c] = 1.0
    return M, cpack, float(c0[0]), float(c0[1]), float(s0)


def _tile128(a, ntiles):
    """[ntiles*128, F] -> [128, ntiles*F] with [p, t*F+f] = a[t*128+p, f]."""
    F = a.shape[1]
    return np.ascontiguousarray(
        a.reshape(ntiles, 128, F).transpose(1, 0, 2).reshape(128, ntiles * F)
    )


def build_nc(K):
    """Build the SPMD 8-core Bass program; returns compiled nc."""
    _, _, c0x, c0y, s0 = host_constants(K)
    inv_s0 = 1.0 / s0
    inv_s0sq = 1.0 / (s0 * s0)

    nc = bacc.Bacc("TRN2", target_bir_lowering=False, debug=False,
                   num_devices=NCORES)

    xt_d = nc.dram_tensor("xt", [128, CB * SH], F32, kind="ExternalInput")
    x_d = nc.dram_tensor("xin", [128, RT * N], F32, kind="ExternalInput")
    m1_d = nc.dram_tensor("m1s", [128, RT * 6], F32, kind="ExternalInput")
    m2_d = nc.dram_tensor("m2t", [128, CB * 6], F32, kind="ExternalInput")
    id_d = nc.dram_tensor("ident", [128, 128], F32, kind="ExternalInput")
    cp_d = nc.dram_tensor("cpack", [9, CPW], F32, kind="ExternalInput")
    out_d = nc.dram_tensor("out", [3, 3], F32, kind="ExternalOutput")

    cp_in = nc.dram_tensor("cp_in", [128, CB * 3], F32)
    cp_out = nc.dram_tensor("cp_out", [NCORES * 128, CB * 3], F32,
                            addr_space="Shared")
    cr_in = nc.dram_tensor("cr_in", [6, 6], F32)
    cr_out = nc.dram_tensor("cr_out", [6, 6], F32, addr_space="Shared")
    stage = nc.dram_tensor("stage", [128], F32)
    mshuf = nc.dram_tensor("mshuf", [81], F32)

    groups = [list(range(NCORES))]

    with tile.TileContext(nc) as tc:
        with (
            tc.tile_pool(name="persist", bufs=1) as pp,
            tc.tile_pool(name="scratch", bufs=2) as sp,
            tc.tile_pool(name="ps_pt", bufs=2, space="PSUM") as ps,
            tc.tile_pool(name="ps_c", bufs=1, space="PSUM") as psc,
        ):
            # ---------- P0: loads (xt first - it gates the AllGather; x
            # right behind - row thresholds + pre-mask must fit in the AG
            # window) ----
            XT = pp.tile([128, CB * SH], F32, tag="XT")
            XCH = 6 * SH
            for c in range(0, CB * SH, XCH):
                nc.sync.dma_start(XT[:, c:c + XCH], xt_d[:, c:c + XCH])
            X = pp.tile([128, RT * N], F32, tag="X")
            for t in range(RT):
                nc.sync.dma_start(X[:, t * N:(t + 1) * N],
                                  x_d[:, t * N:(t + 1) * N])
            cps = pp.tile([9, CPW], F32, tag="cpk")
            nc.sync.dma_start(cps[:], cp_d[:])
            idn = pp.tile([128, 128], F32, tag="idn")
            nc.sync.dma_start(idn[:], id_d[:])
            m2t_s = pp.tile([128, CB * 6], F32, tag="m2")
            nc.sync.dma_start(m2t_s[:], m2_d[:])
            m1t_s = pp.tile([128, RT * 6], F32, tag="m1")
            nc.sync.dma_start(m1t_s[:], m1_d[:])

            # warm scalar-engine activation tables (Copy + Sqrt)
            wrm = sp.tile([1, 2], F32, tag="wrm")
            nc.vector.memset(wrm[:], 1.0)
            wrm2 = sp.tile([1, 2], F32, tag="wrm2")
            nc.scalar.activation(wrm2[:, 0:1], wrm[:, 0:1], AF.Copy)
            nc.scalar.activation(wrm2[:, 1:2], wrm[:, 1:2], AF.Sqrt)
            # all-ones tiles: PE-based partition broadcast/reduce (keeps the
            # Pool engine on its collectives ucode lib - lib swaps cost ~6.5us)
            ones9 = pp.tile([9, 9], F32, tag="ones9")
            nc.vector.memset(ones9[:], 1.0)
            onesr = pp.tile([1, 128], F32, tag="onesr")
            nc.vector.memset(onesr[:], 1.0)

            def XTj(j):
                return XT[:, j * SH:(j + 1) * SH]

            # ---------- P1: column top-3 partials -> AllGather ----------
            c8all = pp.tile([128, CB * 8], F32, tag="c8all")
            for j in range(CB):
                nc.vector.max(out=c8all[:, j * 8:j * 8 + 8], in_=XTj(j))
            c3all = pp.tile([128, CB * 3], F32, tag="c3all")
            nc.vector.tensor_copy(
                c3all[:].rearrange("p (j s) -> p j s", s=3),
                c8all[:].rearrange("p (j s) -> p j s", s=8)[:, :, 0:3])
            nc.scalar.dma_start(cp_in[:], c3all[:])
            nc.gpsimd.collective_compute(
                "AllGather", OP.bypass, replica_groups=groups,
                ins=[cp_in[:]], outs=[cp_out[:]])
            gath = pp.tile([128, NCORES * CB * 3], F32, tag="gath")
            nc.scalar.dma_start(
                gath[:].rearrange("p (k f) -> p k f", k=NCORES),
                cp_out[:].rearrange("(k p) f -> p k f", p=128))

            # ---------- P1b (during AG): row thresholds + row mask -------
            r8 = pp.tile([128, RT * 8], F32, tag="r8")
            for t in range(RT):
                nc.vector.max(out=r8[:, t * 8:t * 8 + 8],
                              in_=X[:, t * N:(t + 1) * N])
            trRow = pp.tile([1, SH], F32, tag="trRow")
            for t in range(RT):
                ptr = ps.tile([1, 128], F32, tag="tps")
                nc.tensor.transpose(ptr[:], r8[:, t * 8 + 2:t * 8 + 3], idn[:])
                nc.scalar.activation(trRow[:, t * 128:(t + 1) * 128], ptr[:],
                                     AF.Copy)
            trRow2 = pp.tile([1, SH], F32, tag="trRow2")
            nc.vector.tensor_scalar_max(trRow2[:], trRow[:], T0)
            # broadcast across partitions via PE ones-matmul
            trBp = psc.tile([128, SH], F32, tag="trBp")
            nc.tensor.matmul(trBp[:], onesr[:], trRow2[:], start=True,
                             stop=True)
            trB = pp.tile([128, SH], F32, tag="trB")
            nc.scalar.activation(trB[:], trBp[:], AF.Copy)

            # pre-mask: w = XT * (XT >= trB)   (in place, DVE only - Pool
            # tensor ops would force ucode lib swaps around the collectives).
            # 4 big chunks per pass: per-instruction overhead ~280ns, so few
            # wide ops beat 24 per-block ones.
            geb = pp.tile([128, CB * SH], F32, tag="geb")
            trBv = trB[:].unsqueeze(1).to_broadcast([128, 6, SH])
            MCH = 6 * SH
            for c in range(0, CB * SH, MCH):
                nc.vector.tensor_tensor(
                    geb[:, c:c + MCH].rearrange("p (b f) -> p b f", f=SH),
                    XT[:, c:c + MCH].rearrange("p (b f) -> p b f", f=SH),
                    trBv, OP.is_ge)
            for c in range(0, CB * SH, MCH):
                nc.vector.tensor_tensor(XT[:, c:c + MCH], XT[:, c:c + MCH],
                                        geb[:, c:c + MCH], OP.mult)

            # ---------- P2: post-AG combine + fused col mask + Gram ------
            # interleave per block so the PE can start on block 0 ASAP
            cm8 = pp.tile([128, CB * 8], F32, tag="cm8")
            gv = gath[:].rearrange("p (k j s) -> p j k s", k=NCORES, s=3)
            psB = psc.tile([6, SH], F32, tag="psB")
            for j in range(CB):
                nc.vector.max(out=cm8[:, j * 8:j * 8 + 8], in_=gv[:, j])
                nc.vector.scalar_tensor_tensor(XTj(j), XTj(j),
                                               cm8[:, j * 8 + 2:j * 8 + 3],
                                               XTj(j), OP.is_ge, OP.mult)
                nc.tensor.matmul(psB[:], m2t_s[:, j * 6:(j + 1) * 6], XTj(j),
                                 start=(j == 0), stop=(j == CB - 1))
            BtS = sp.tile([6, SH], F32, tag="BtS")
            nc.scalar.activation(BtS[:], psB[:], AF.Copy)
            Bcol = sp.tile([128, RT * 6], F32, tag="Bcol")
            for t in range(RT):
                pt = ps.tile([128, 6], F32, tag="tps")
                nc.tensor.transpose(pt[:], BtS[:, t * 128:(t + 1) * 128],
                                    idn[0:6, 0:6])
                nc.scalar.activation(Bcol[:, t * 6:(t + 1) * 6], pt[:], AF.Copy)
            psC = psc.tile([6, 6], F32, tag="psC")
            for t in range(RT):
                nc.tensor.matmul(psC[:], m1t_s[:, t * 6:(t + 1) * 6],
                                 Bcol[:, t * 6:(t + 1) * 6],
                                 start=(t == 0), stop=(t == RT - 1))
            Cp = sp.tile([6, 6], F32, tag="Cp")
            nc.scalar.activation(Cp[:], psC[:], AF.Copy)
            nc.sync.dma_start(cr_in[:], Cp[:])
            nc.gpsimd.collective_compute(
                "AllReduce", OP.add, replica_groups=groups,
                ins=[cr_in[:]], outs=[cr_out[:]])

            # ---------- tail ----------
            _tail(nc, pp, sp, ps, cps, idn, cr_out, stage, mshuf, out_d,
                  c0x, c0y, inv_s0, inv_s0sq, ones9)

    nc.compile()
    return nc


def _transpose(nc, ps, sp, in_sb, n, idn, tag):
    """PE-transpose square [n, n] SBUF -> new SBUF tile."""
    pt = ps.tile([n, n], F32, tag="tps")
    nc.tensor.transpose(pt[:], in_sb, idn[:n, :n])
    ot = sp.tile([n, n], F32, tag=f"ot_{tag}")
    nc.scalar.activation(ot[:], pt[:], AF.Copy)
    return ot


def _pow50(nc, ps, sp, m_sb, n, tag):
    """Direction of M^50 v via rescaled squarings M <- 2*(M@M);
    M50 = 2*((2*(M32@M16)) @ M2). All operands feed normalized eigvecs."""
    powers = {}
    cur = m_sb
    for i in range(1, 6):  # M2, M4, M8, M16, M32
        pm = ps.tile([n, n], F32, tag="tps")
        nc.tensor.matmul(pm[:], cur, cur, start=True, stop=True)
        nxt = sp.tile([n, n], F32, tag=f"pws_{tag}_{i}")
        nc.vector.tensor_scalar_mul(nxt[:], pm[:], 2.0)
        powers[2 ** i] = nxt
        cur = nxt[:]
    pm = ps.tile([n, n], F32, tag="tps")
    nc.tensor.matmul(pm[:], powers[32][:], powers[16][:], start=True, stop=True)
    m48 = sp.tile([n, n], F32, tag=f"pws_{tag}_48")
    nc.vector.tensor_scalar_mul(m48[:], pm[:], 2.0)
    pm = ps.tile([n, n], F32, tag="tps")
    nc.tensor.matmul(pm[:], m48[:], powers[2][:], start=True, stop=True)
    m50 = sp.tile([n, n], F32, tag=f"pws_{tag}_50")
    nc.vector.tensor_scalar_mul(m50[:], pm[:], 2.0)
    return m50


def _tail(nc, pp, sp, ps, cps, idn, cr_out, stage, mshuf, out_d,
          c0x, c0y, inv_s0, inv_s0sq, ones9):
    """C' (6x6 pre-centered Gram) -> Hartley -> L transforms -> Mmat ->
    power chains -> projection -> out."""
    e5 = cps[0:6, C_E5:C_E5 + 1]
    i9h = cps[0:9, C_I9H:C_I9H + 9]
    et69 = cps[0:6, C_ET69:C_ET69 + 9]
    i3c = cps[0:3, C_I3:C_I3 + 3]
    v09 = cps[0:9, C_V09:C_V09 + 1]
    v06 = cps[0:6, C_V06:C_V06 + 1]
    sel1 = cps[0:3, C_SEL1:C_SEL1 + 6]
    sel2 = cps[0:3, C_SEL2:C_SEL2 + 6]

    Cr = sp.tile([6, 6], F32, tag="Cr")
    nc.sync.dma_start(Cr[:], cr_out[:])
    CrT = _transpose(nc, ps, sp, Cr[:], 6, idn, "crt")

    sc = pp.tile([128, 96], F32, tag="tailsc")

    def scv(a, b):
        return sc[0:1, a:b]

    mo_ps = ps.tile([1, 6], F32, tag="tps")
    nc.tensor.matmul(mo_ps[:], e5, CrT[:], start=True, stop=True)
    nc.scalar.activation(scv(0, 6), mo_ps[:], AF.Copy)    # side1 moments
    mo_ps2 = ps.tile([1, 6], F32, tag="tps")
    nc.tensor.matmul(mo_ps2[:], e5, Cr[:], start=True, stop=True)
    nc.scalar.activation(scv(6, 12), mo_ps2[:], AF.Copy)  # side2 moments

    def pair(k):  # element k of each side: free idxs (k, k+6)
        return sc[0:1, 0:12].rearrange("p (g d) -> p d g", g=2)[:, k, :]

    # moments per side: [Sxx, Sxy, Sx, Syy, Sy, Sw]  (pre-centered coords)
    Sxx, Sx, Syy, Sy, Sw = pair(0), pair(2), pair(3), pair(4), pair(5)
    ws = scv(12, 14); nc.vector.tensor_scalar_add(ws, Sw, EPS)
    rws = scv(14, 16); nc.vector.reciprocal(rws, ws)
    cx = scv(16, 18); nc.vector.tensor_tensor(cx, Sx, rws, OP.mult)
    cy = scv(18, 20); nc.vector.tensor_tensor(cy, Sy, rws, OP.mult)
    t_a = scv(20, 22); nc.vector.tensor_tensor(t_a, cx, Sx, OP.mult)
    t_b = scv(22, 24); nc.vector.tensor_tensor(t_b, cy, Sy, OP.mult)
    cdS = scv(24, 26); nc.vector.tensor_tensor(cdS, t_a, t_b, OP.add)
    u_a = scv(26, 28); nc.vector.tensor_tensor(u_a, cx, cx, OP.mult)
    u_b = scv(28, 30); nc.vector.tensor_tensor(u_b, cy, cy, OP.mult)
    c2_ = scv(30, 32); nc.vector.tensor_tensor(c2_, u_a, u_b, OP.add)
    sq_ = scv(32, 34); nc.vector.tensor_tensor(sq_, Sxx, Syy, OP.add)
    n2c = scv(34, 36); nc.vector.tensor_scalar_mul(n2c, cdS, -2.0)
    c2w = scv(36, 38); nc.vector.tensor_tensor(c2w, c2_, Sw, OP.mult)
    m_ = scv(38, 40); nc.vector.tensor_tensor(m_, sq_, n2c, OP.add)
    m2_ = scv(40, 42); nc.vector.tensor_tensor(m2_, m_, c2w, OP.add)
    md2 = scv(42, 44); nc.vector.tensor_tensor(md2, m2_, rws, OP.mult)
    # md in ORIGINAL units: md2 * inv_s0^2 + EPS, then sqrt
    md2e = scv(44, 46)
    nc.vector.tensor_scalar(md2e, md2, inv_s0sq, EPS, OP.mult, OP.add)
    md = scv(46, 48); nc.scalar.activation(md, md2e, AF.Sqrt)
    mde = scv(48, 50); nc.vector.tensor_scalar_add(mde, md, EPS)
    rmd = scv(50, 52); nc.vector.reciprocal(rmd, mde)
    s_ = scv(52, 54); nc.vector.tensor_scalar_mul(s_, rmd, SQRT2)
    # hartley scale in pre-centered units; L values with signs folded in:
    # macx = -a*cx, na2cx = -a^2*cx, m2a2cx = -2a^2*cx, squares sign-free
    a_ = scv(54, 56); nc.vector.tensor_scalar_mul(a_, s_, inv_s0)
    na = scv(56, 58); nc.vector.tensor_scalar_mul(na, a_, -1.0)
    macx = scv(58, 60); nc.vector.tensor_tensor(macx, na, cx, OP.mult)
    macy = scv(60, 62); nc.vector.tensor_tensor(macy, na, cy, OP.mult)
    a2 = scv(62, 64); nc.vector.tensor_tensor(a2, a_, a_, OP.mult)
    na2cx = scv(64, 66); nc.vector.tensor_tensor(na2cx, a_, macx, OP.mult)
    na2cy = scv(66, 68); nc.vector.tensor_tensor(na2cy, a_, macy, OP.mult)
    m2a2cx = scv(68, 70); nc.vector.tensor_scalar_mul(m2a2cx, na2cx, 2.0)
    m2a2cy = scv(70, 72); nc.vector.tensor_scalar_mul(m2a2cy, na2cy, 2.0)
    a2cx2 = scv(72, 74); nc.vector.tensor_tensor(a2cx2, macx, macx, OP.mult)
    a2cy2 = scv(74, 76); nc.vector.tensor_tensor(a2cy2, macy, macy, OP.mult)
    a2cxcy = scv(76, 78); nc.vector.tensor_tensor(a2cxcy, macx, macy, OP.mult)
    # T entries (original units): cx_o = cx*inv_s0 + c0x ; nscx = -s*cx_o
    cxo = scv(78, 80)
    nc.vector.tensor_scalar(cxo, cx, inv_s0, c0x, OP.mult, OP.add)
    cyo = scv(80, 82)
    nc.vector.tensor_scalar(cyo, cy, inv_s0, c0y, OP.mult, OP.add)
    ns = scv(82, 84); nc.vector.tensor_scalar_mul(ns, s_, -1.0)
    nscx = scv(84, 86); nc.vector.tensor_tensor(nscx, ns, cxo, OP.mult)
    nscy = scv(86, 88); nc.vector.tensor_tensor(nscy, ns, cyo, OP.mult)

    # ----- build L1^T|L2^T and T1|T2 tiles directly: replicate the scalar
    # row across partitions with a ones-matmul, then accumulate constant
    # position-masks scaled by per-partition scalars (no DRAM bounce).
    rpps = ps.tile([6, 36], F32, tag="tps")
    nc.tensor.matmul(rpps[:], ones9[0:1, 0:6], sc[0:1, 52:88],
                     start=True, stop=True)
    rep6 = sp.tile([6, 36], F32, tag="rep6")
    nc.vector.tensor_copy(rep6[:], rpps[:])

    LT = sp.tile([6, 12], F32, tag="LT")   # [:, 0:6] = L1^T, [:, 6:12] = L2^T
    for side in range(2):
        dst = LT[:, side * 6:(side + 1) * 6]
        for k, (_, off, _) in enumerate(LMASKS):
            msk = cps[0:6, C_LM + 6 * k:C_LM + 6 * k + 6]
            if off is None:
                nc.vector.tensor_tensor(dst, dst, msk, OP.add)
            else:
                col = rep6[:, off - 52 + side:off - 52 + side + 1]
                if k == 0:
                    nc.vector.tensor_scalar_mul(dst, msk, col)
                else:
                    nc.vector.scalar_tensor_tensor(dst, msk, col, dst,
                                                   OP.mult, OP.add)
    TT = sp.tile([3, 6], F32, tag="TT")    # [:, 0:3] = T1, [:, 3:6] = T2
    for side in range(2):
        dst = TT[:, side * 3:(side + 1) * 3]
        for t, (_, off, _) in enumerate(TMASKS):
            msk = cps[0:3, C_TM + 3 * t:C_TM + 3 * t + 3]
            if off is None:
                nc.vector.tensor_tensor(dst, dst, msk, OP.add)
            else:
                col = rep6[0:3, off - 52 + side:off - 52 + side + 1]
                if t == 0:
                    nc.vector.tensor_scalar_mul(dst, msk, col)
                else:
                    nc.vector.scalar_tensor_tensor(dst, msk, col, dst,
                                                   OP.mult, OP.add)

    # ----- C2^T = L2 C'^T L1^T ; then G2 = E C2 E^T --------------------
    u2ps = ps.tile([6, 6], F32, tag="tps")
    nc.tensor.matmul(u2ps[:], LT[:, 6:12], CrT[:], start=True, stop=True)
    U2s = sp.tile([6, 6], F32, tag="U2s")
    nc.scalar.activation(U2s[:], u2ps[:], AF.Copy)
    U2T = _transpose(nc, ps, sp, U2s[:], 6, idn, "u2t")
    c2ps = ps.tile([6, 6], F32, tag="tps")
    nc.tensor.matmul(c2ps[:], U2T[:], LT[:, 0:6], start=True, stop=True)
    C2T = sp.tile([6, 6], F32, tag="C2T")
    nc.scalar.activation(C2T[:], c2ps[:], AF.Copy)

    z_ps = ps.tile([6, 9], F32, tag="tps")
    nc.tensor.matmul(z_ps[:], C2T[:], et69, start=True, stop=True)  # C2 E^T
    Zs = sp.tile([6, 9], F32, tag="Zs")
    nc.scalar.activation(Zs[:], z_ps[:], AF.Copy)
    g_ps = ps.tile([9, 9], F32, tag="tps")
    nc.tensor.matmul(g_ps[:], et69, Zs[:], start=True, stop=True)   # E @ Z
    G2 = sp.tile([9, 9], F32, tag="G2")
    nc.scalar.activation(G2[:], g_ps[:], AF.Copy)

    # Mmat[3p+q, 3r+s] = G2[3p+r, 3q+s]: bounce via DRAM, 3 parallel reads
    nc.sync.dma_start(mshuf[:], G2[:])
    Mmat = sp.tile([9, 9], F32, tag="Mmat")
    for p, eng in zip(range(3), (nc.sync, nc.scalar, nc.sync)):
        eng.dma_start(
            Mmat[3 * p:3 * p + 3, :].rearrange("q (r s) -> q r s", s=3),
            mshuf[:].rearrange("(p q1 r s) -> p q1 r s", p=3, q1=3, r=3)
            .transpose([0, 2, 1, 3])[p])

    # shifted scaled 9x9: Msp = Mmat/(2 lam) - I/2 (sign irrelevant, even pow)
    dg = sp.tile([9, 9], F32, tag="dg")
    nc.vector.tensor_tensor(dg[:], Mmat[:], i9h, OP.mult)  # diag/2
    lam2 = sp.tile([9, 1], F32, tag="lam2")
    nc.vector.tensor_reduce(lam2[:], dg[:], AX.X, OP.add)
    l2ps = ps.tile([9, 1], F32, tag="tps")
    nc.tensor.matmul(l2ps[:], ones9[:], lam2[:], start=True, stop=True)
    lam2r = sp.tile([9, 1], F32, tag="lam2r")
    nc.vector.tensor_copy(lam2r[:], l2ps[:])
    lam4 = sp.tile([9, 1], F32, tag="lam4")
    nc.vector.tensor_scalar_mul(lam4[:], lam2r[:], 4.0)  # = 2*lam
    inv2l = sp.tile([9, 1], F32, tag="inv2l")
    nc.vector.reciprocal(inv2l[:], lam4[:])
    Msp = sp.tile([9, 9], F32, tag="Msp")
    nc.vector.scalar_tensor_tensor(Msp[:], Mmat[:], inv2l[:], i9h,
                                   OP.mult, OP.subtract)
    M50 = _pow50(nc, ps, sp, Msp[:], 9, "m9")

    # w9 left UNNORMALIZED: E scales by ||w9||; all downstream eigvec math is
    # scale-free, only the final column scaling needs a 1/||w9|| fix, which
    # is computed here off the critical path and folded into f2 at the end.
    w9ps = ps.tile([1, 9], F32, tag="tps")
    nc.tensor.matmul(w9ps[:], v09, M50[:], start=True, stop=True)
    w9 = sp.tile([1, 9], F32, tag="w9")
    nc.vector.tensor_copy(w9[:], w9ps[:])
    w9sq = sp.tile([1, 9], F32, tag="w9sq")
    nc.vector.tensor_tensor(w9sq[:], w9[:], w9[:], OP.mult)
    nn9 = sp.tile([1, 1], F32, tag="nn9")
    nc.vector.tensor_reduce(nn9[:], w9sq[:], AX.X, OP.add)
    sr9 = sp.tile([1, 1], F32, tag="sr9")
    nc.scalar.activation(sr9[:], nn9[:], AF.Sqrt)
    rs9 = sp.tile([1, 1], F32, tag="rs9")
    nc.vector.reciprocal(rs9[:], sr9[:])
    r9ps = ps.tile([2, 1], F32, tag="tps")
    nc.tensor.matmul(r9ps[:], ones9[0:1, 0:2], rs9[:], start=True, stop=True)
    rs9b = sp.tile([2, 1], F32, tag="rs9b")
    nc.vector.tensor_copy(rs9b[:], r9ps[:])

    # E_raw^T via 3 tiny PE transposes (no DRAM bounce), then
    # E = T2^T (E_raw T1):  Y = mm(ETraw, T1) = E_raw T1 ; Es = mm(T2, Y)
    ETraw = sp.tile([3, 3], F32, tag="ETraw")
    for i in range(3):
        pt3 = ps.tile([3, 1], F32, tag="tps")
        nc.tensor.transpose(pt3[:], w9[0:1, 3 * i:3 * i + 3], idn[0:1, 0:1])
        nc.scalar.activation(ETraw[:, i:i + 1], pt3[:], AF.Copy)
    yps = ps.tile([3, 3], F32, tag="tps")
    nc.tensor.matmul(yps[:], ETraw[:], TT[:, 0:3], start=True, stop=True)
    Ys = sp.tile([3, 3], F32, tag="Ys")
    nc.vector.tensor_copy(Ys[:], yps[:])
    eps_ = ps.tile([3, 3], F32, tag="tps")
    nc.tensor.matmul(eps_[:], TT[:, 3:6], Ys[:], start=True, stop=True)
    Es = sp.tile([3, 3], F32, tag="Es")
    nc.vector.tensor_copy(Es[:], eps_[:])
    ETs = _transpose(nc, ps, sp, Es[:], 3, idn, "ets")

    # B = E^T E ; blockdiag 6x6 chain for v1 (max) and v3 (min)
    bps = ps.tile([3, 3], F32, tag="tps")
    nc.tensor.matmul(bps[:], Es[:], Es[:], start=True, stop=True)
    Bm = sp.tile([3, 3], F32, tag="Bm")
    nc.scalar.activation(Bm[:], bps[:], AF.Copy)
    dg3 = sp.tile([3, 3], F32, tag="dg3")
    nc.vector.tensor_tensor(dg3[:], Bm[:], i3c, OP.mult)
    lb = sp.tile([3, 1], F32, tag="lb")
    nc.vector.tensor_reduce(lb[:], dg3[:], AX.X, OP.add)
    lbps = ps.tile([3, 1], F32, tag="tps")
    nc.tensor.matmul(lbps[:], ones9[0:3, 0:3], lb[:], start=True, stop=True)
    lbr = sp.tile([3, 1], F32, tag="lbr")
    nc.vector.tensor_copy(lbr[:], lbps[:])
    invlb = sp.tile([3, 1], F32, tag="invlb")
    nc.vector.reciprocal(invlb[:], lbr[:])
    Bs3 = sp.tile([3, 3], F32, tag="Bs3")
    nc.vector.tensor_scalar_mul(Bs3[:], Bm[:], invlb[:])
    IB = sp.tile([3, 3], F32, tag="IB")
    nc.vector.tensor_tensor(IB[:], i3c, Bs3[:], OP.subtract)
    bdps = ps.tile([6, 6], F32, tag="tps")
    nc.tensor.matmul(bdps[:, 0:3], sel1, Bs3[:], start=True, stop=True)
    nc.tensor.matmul(bdps[:, 3:6], sel2, IB[:], start=True, stop=True)
    BD = sp.tile([6, 6], F32, tag="BD")
    nc.scalar.activation(BD[:], bdps[:], AF.Copy)
    BD50 = _pow50(nc, ps, sp, BD[:], 6, "m6")

    w6ps = ps.tile([1, 6], F32, tag="tps")
    nc.tensor.matmul(w6ps[:], v06, BD50[:], start=True, stop=True)
    w6 = sp.tile([1, 6], F32, tag="w6")
    nc.scalar.activation(w6[:], w6ps[:], AF.Copy)
    w6sq = sp.tile([1, 6], F32, tag="w6sq")
    nc.vector.tensor_tensor(w6sq[:], w6[:], w6[:], OP.mult)
    nn6 = sp.tile([1, 2], F32, tag="nn6")
    nc.vector.tensor_reduce(nn6[:].unsqueeze(2),
                            w6sq[:].rearrange("p (g d) -> p g d", g=2), AX.X,
                            OP.add)
    sr6 = sp.tile([1, 2], F32, tag="sr6")
    nc.scalar.activation(sr6[:], nn6[:], AF.Sqrt)
    rs6 = sp.tile([1, 2], F32, tag="rs6")
    nc.vector.reciprocal(rs6[:], sr6[:])
    vv = sp.tile([1, 6], F32, tag="vv")
    nc.vector.tensor_tensor(
        vv[:].rearrange("p (g d) -> p g d", g=2),
        w6[:].rearrange("p (g d) -> p g d", g=2),
        rs6[:].unsqueeze(2).to_broadcast([1, 2, 3]), OP.mult)

    # v2 = cross(v3, v1), normalized with EPS (as reference)
    aa = sp.tile([1, 6], F32, tag="aa")
    nc.vector.tensor_copy(
        aa[:].rearrange("p (r d) -> p r d", r=2),
        vv[:, 3:6].unsqueeze(1).to_broadcast([1, 2, 3]))
    bb = sp.tile([1, 6], F32, tag="bb")
    nc.vector.tensor_copy(
        bb[:].rearrange("p (r d) -> p r d", r=2),
        vv[:, 0:3].unsqueeze(1).to_broadcast([1, 2, 3]))
    cr1 = sp.tile([1, 3], F32, tag="cr1")
    nc.vector.tensor_tensor(cr1[:], aa[:, 1:4], bb[:, 2:5], OP.mult)
    cr2 = sp.tile([1, 3], F32, tag="cr2")
    nc.vector.tensor_tensor(cr2[:], aa[:, 2:5], bb[:, 1:4], OP.mult)
    v2r = sp.tile([1, 3], F32, tag="v2r")
    nc.vector.tensor_tensor(v2r[:], cr1[:], cr2[:], OP.subtract)
    v2sq = sp.tile([1, 3], F32, tag="v2sq")
    nc.vector.tensor_tensor(v2sq[:], v2r[:], v2r[:], OP.mult)
    nn2 = sp.tile([1, 1], F32, tag="nn2")
    nc.vector.tensor_reduce(nn2[:], v2sq[:], AX.X, OP.add)
    sr2 = sp.tile([1, 1], F32, tag="sr2")
    nc.scalar.activation(sr2[:], nn2[:], AF.Sqrt)
    sr2e = sp.tile([1, 1], F32, tag="sr2e")
    nc.vector.tensor_scalar_add(sr2e[:], sr2[:], EPS)
    rs2 = sp.tile([1, 1], F32, tag="rs2")
    nc.vector.reciprocal(rs2[:], sr2e[:])
    v2 = sp.tile([1, 3], F32, tag="v2")
    nc.vector.tensor_tensor(v2[:], v2r[:], rs2[:].to_broadcast([1, 3]), OP.mult)

    # V columns/rows via tiny PE transposes (no DRAM bounce)
    Vc = sp.tile([3, 2], F32, tag="Vc")
    ptv = ps.tile([3, 1], F32, tag="tps")
    nc.tensor.transpose(ptv[:], vv[0:1, 0:3], idn[0:1, 0:1])
    nc.scalar.activation(Vc[:, 0:1], ptv[:], AF.Copy)
    ptv2 = ps.tile([3, 1], F32, tag="tps")
    nc.tensor.transpose(ptv2[:], v2[0:1, 0:3], idn[0:1, 0:1])
    nc.scalar.activation(Vc[:, 1:2], ptv2[:], AF.Copy)
    ptvr = ps.tile([2, 3], F32, tag="tps")
    nc.tensor.transpose(ptvr[:], Vc[:], idn[0:3, 0:3])
    Vr = sp.tile([2, 3], F32, tag="Vr")
    nc.vector.tensor_copy(Vr[:], ptvr[:])
    evps = ps.tile([2, 3], F32, tag="tps")
    nc.tensor.matmul(evps[:], Vc[:], ETs[:], start=True, stop=True)
    Evr = sp.tile([2, 3], F32, tag="Evr")
    nc.scalar.activation(Evr[:], evps[:], AF.Copy)
    evsq = sp.tile([2, 3], F32, tag="evsq")
    nc.vector.tensor_tensor(evsq[:], Evr[:], Evr[:], OP.mult)
    ss2 = sp.tile([2, 1], F32, tag="ss2")
    nc.vector.tensor_reduce(ss2[:], evsq[:], AX.X, OP.add)
    sv = sp.tile([2, 1], F32, tag="sv")
    nc.scalar.activation(sv[:], ss2[:], AF.Sqrt)
    ssps = ps.tile([2, 1], F32, tag="tps")
    nc.tensor.matmul(ssps[:], ones9[0:2, 0:2], sv[:], start=True, stop=True)
    ssum = sp.tile([2, 1], F32, tag="ssum")
    nc.vector.tensor_copy(ssum[:], ssps[:])
    savg = sp.tile([2, 1], F32, tag="savg")
    nc.vector.tensor_scalar_mul(savg[:], ssum[:], 0.5)
    sve = sp.tile([2, 1], F32, tag="sve")
    nc.vector.tensor_scalar_add(sve[:], sv[:], EPS)
    rsv = sp.tile([2, 1], F32, tag="rsv")
    nc.vector.reciprocal(rsv[:], sve[:])
    f2 = sp.tile([2, 1], F32, tag="f2")
    nc.vector.tensor_tensor(f2[:], rsv[:], savg[:], OP.mult)
    f2n = sp.tile([2, 1], F32, tag="f2n")
    nc.vector.tensor_tensor(f2n[:], f2[:], rs9b[:], OP.mult)  # 1/||w9|| fix
    U2 = sp.tile([2, 3], F32, tag="U2")
    nc.vector.tensor_scalar_mul(U2[:], Evr[:], f2n[:])
    ops_ = ps.tile([3, 3], F32, tag="tps")
    nc.tensor.matmul(ops_[:], U2[:], Vr[:], start=True, stop=True)
    outs = sp.tile([3, 3], F32, tag="outs")
    nc.scalar.activation(outs[:], ops_[:], AF.Copy)
    nc.sync.dma_start(out_d[:], outs[:])


def make_in_maps(P, K):
    """Host-side shard + constant prep: list of 8 input dicts."""
    P = np.asarray(P, np.float32)
    K = np.asarray(K, np.float32)
    Pc = np.ascontiguousarray(P[:N, :N])
    M, cpack, _, _, _ = host_constants(K)
    m2t = _tile128(M, CB)
    ident = np.eye(128, dtype=np.float32)
    in_maps = []
    for k in range(NCORES):
        sh = Pc[k * SH:(k + 1) * SH]
        shT = np.ascontiguousarray(sh.T)          # [3072 cols, 384 rows]
        in_maps.append({
            "xt": _tile128(shT, CB),
            "xin": _tile128(sh, RT),
            "m1s": _tile128(M[k * SH:(k + 1) * SH], RT),
            "m2t": m2t,
            "ident": ident,
            "cpack": cpack,
        })
    return in_maps


_NC_CACHE = {}


def kernel(P, K):
    from concourse.bass_utils import run_bass_kernel_spmd
    key = (np.asarray(P).shape, np.asarray(K, np.float32).tobytes())
    if key not in _NC_CACHE:
        _NC_CACHE[key] = build_nc(K)
    nc = _NC_CACHE[key]
    in_maps = make_in_maps(P, K)
    res = run_bass_kernel_spmd(nc, in_maps, core_ids=list(range(NCORES)))
    return np.asarray(res.results[0]["out"], np.float32)
